# revision 6
# baseline (speedup 1.0000x reference)
"""Trainium2 Bass kernel for nn_CrossAttention (packed cross-attention).

Math (verified against the jax reference):
  The reference scatters packed rows into dense slots, runs masked dense
  attention over T*N tokens, and gathers pred rows back.  Because q is zero
  in ctx slots, k/v are zero in pred slots, and (pred x pred) pairs are
  masked to -inf, this is exactly: for each batch b, the packed pred rows
  cross-attend to the packed ctx rows of the same batch:

    Q = Xp_b @ Wq ; [K|V] = Xc_b @ Wkv          (Xp_b, Xc_b: [1024, 512])
    out_b = concat_h( softmax(Q_h K_h^T / 8) V_h ) @ Wproj + bproj

  Softmax needs no max-subtraction: |scores| < ~7 (verified), exp is safe
  in fp32.

Sharding: 8 cores = (2 batches) x (4 head-pairs).  Each core computes two
heads of one batch and the partial output projection for those heads
(row-sharded Wproj); the host sums the 4 partials per batch and adds bproj.

v2 design (v1 measured 58.5us; see git-less changelog in comments):
  - input DMA split across BOTH hwdge queues (ACT queue starts ~2.4us,
    SP ~5.3us) so x slabs land ~2x faster; consumers accumulate in
    arrival order
  - attention runs HEAD-SEQUENTIAL (all kt of h0, then h1) with a single
    shared PV psum tile; head 0's softmax-normalizer tail (reciprocal,
    broadcast, multiply) overlaps head 1's exp stream on the otherwise
    idle DVE, so only head 1's short tail remains at the end
  - 1/Z via DVE reciprocal_approx_fast (18-bit) instead of ACT Ln/Exp:
    keeps the ACT engine 100%-dedicated to the 16-tile exp stream, which
    is the per-core floor (2.1M exps / 128 lanes / 1.2GHz = 13.7us)
  - all constant/zero-pad memsets moved to the idle GpSimd(Pool) engine
  - exp stream software-pipelined one S^T tile ahead (as v1); p_t ring
    deepened to 6 so PV may lag behind exp while pv drains the prev head
  - PSUM: qkv pools close before attention pools open; s_t(4 banks) +
    pv(2) + rbc(2) = 8; out_ps reuses s_t banks after the last exp
"""

import sys

if "/opt/trn_rl_repo" not in sys.path:
    sys.path.insert(0, "/opt/trn_rl_repo")

import numpy as np

B, T, N, C, H = 2, 8, 256, 512, 8
T_CTX = T // 2
HD = C // H            # 64
SEQ = T_CTX * N        # 1024 packed tokens per batch (q and kv)
NCORE = 8
CT_N = C // 128        # 4 contraction tiles over C
KT_N = SEQ // 128      # 8 key tiles
SCALE = HD ** -0.5

_PROG = None
SPLIT_WAITS = True  # walrus needs it; CoreSim chokes on it
USE_RECIP_APPROX = False  # custom-DVE op; flip on if the axon hook supports it


def _build_program():
    import concourse.bass as bass
    import concourse.tile as tile
    from concourse import mybir

    class TrimTailTileContext(tile.TileContext):
        """Skip the second end-of-kernel all-engine barrier: executions of
        the NEFF are serialized by the runtime, and the semaphore clear is
        still ordered after the first barrier on the gpsimd queue."""

        def _drain_and_barrier(self, tick_clock, wait_clock):
            from concourse.vector_clock import ScopedClock

            drain_inst = self.nc.sync.drain()
            wait_clock.add_sem_waits(
                drain_inst.ins, ScopedClock({None: tick_clock.global_clock}))
            self.nc.all_engine_barrier()
            popped = self.nc._tile_sem_poison_stack.pop()
            assert popped is self._sem_poison
            self.nc.clear_and_free_semaphores(
                list(self.sems.allocated().values()))

    F16 = mybir.dt.float16

    nc = bass.Bass("TRN2", target_bir_lowering=False, debug=False,
                   num_devices=NCORE)

    xpT = nc.dram_tensor("xpT", [C, SEQ], F16, kind="ExternalInput").ap()
    xcT = nc.dram_tensor("xcT", [C, SEQ], F16, kind="ExternalInput").ap()
    wq = nc.dram_tensor("wq", [C, 128], F16, kind="ExternalInput").ap()
    wk = nc.dram_tensor("wk", [C, 128], F16, kind="ExternalInput").ap()
    wv = nc.dram_tensor("wv", [C, 128], F16, kind="ExternalInput").ap()
    wp = nc.dram_tensor("wp", [128, C], F16, kind="ExternalInput").ap()
    out = nc.dram_tensor("out", [SEQ, C], F16, kind="ExternalOutput").ap()

    with TrimTailTileContext(nc) as tc:
        _emit(nc, tc, mybir, xpT, xcT, wq, wk, wv, wp, out)
    if SPLIT_WAITS:
        _split_sync_waits(nc, mybir)
    return nc


def _split_sync_waits(nc, mybir):
    """This container's walrus build has tight per-instruction sync-wait
    limits ("Too many sync wait commands": Matmult holds 1 wait command,
    control-class instructions 2).  Tile freely assigns more.  Rewrite each
    block, moving overflow waits onto same-engine NoOps inserted directly
    before the over-limit instruction (safe: the engine queue executes in
    order, so the waits still complete before the instruction runs)."""
    LIMITS = {}
    DEFAULT = 1
    NOP_W = 1
    n = 0
    for fn in nc.m.functions:
        for bb in fn.blocks:
            insts = bb.instructions
            new = []
            changed = False
            for inst in insts:
                si = inst.sync_info
                waits = list(si.on_wait) if si is not None else []
                limit = LIMITS.get(inst.opcode, DEFAULT)
                if len(waits) > limit:
                    extra = waits[:-limit] if limit else waits
                    keep = waits[-limit:] if limit else []
                    # the end-of-kernel drain carries one wait per logical
                    # processor; spread its nops across engines so they
                    # retire in parallel (the following barrier re-syncs),
                    # instead of ~130ns each serially on the sync sequencer
                    if inst.opcode == "Drain" and len(extra) > 4:
                        engs = [mybir.EngineType.SP, mybir.EngineType.PE,
                                mybir.EngineType.DVE,
                                mybir.EngineType.Activation,
                                mybir.EngineType.Pool]
                    else:
                        engs = [inst.engine]
                    for i in range(0, len(extra), NOP_W):
                        nop = mybir.InstNoOp(
                            name=f"I-waitsplit-{n}", ins=[], outs=[],
                            engine=engs[(i // NOP_W) % len(engs)],
                            sync_info=mybir.SyncInfo(
                                on_wait=extra[i:i + NOP_W], on_update=[]))
                        new.append(nop)
                        n += 1
                    inst.sync_info = mybir.SyncInfo(
                        on_wait=keep, on_update=list(si.on_update))
                    changed = True
                new.append(inst)
            if changed:
                bb.instructions = new


def _emit(nc, tc, mybir, xpT, xcT, wq, wk, wv, wp, out):
    from contextlib import ExitStack

    F32 = mybir.dt.float32
    F16 = mybir.dt.float16
    Exp = mybir.ActivationFunctionType.Exp

    P_DEPTH = 6

    with ExitStack() as ctx:
        sb = ctx.enter_context(tc.tile_pool(name="sb", bufs=1))

        # separate tiles per DMA chunk / per column half: Tile tracks
        # dependencies at tile granularity, so consumers must not share a
        # tile with unrelated later writes
        xp_sb = [sb.tile([128, SEQ], F16, tag=f"xp{ct}", name=f"xp{ct}")
                 for ct in range(CT_N)]
        xc_sb = [sb.tile([128, SEQ], F16, tag=f"xc{ct}", name=f"xc{ct}")
                 for ct in range(CT_N)]
        wq_sb = sb.tile([128, CT_N, 128], F16, tag="wq")
        wk_sb = sb.tile([128, CT_N, 128], F16, tag="wk")
        wv_sb = sb.tile([128, CT_N, 128], F16, tag="wv")
        wp0_sb = sb.tile([128, C], F16, tag="wp0")
        wp1_sb = sb.tile([128, C], F16, tag="wp1")
        qt_p = [sb.tile([128, SEQ], F16, tag=f"qt{h}", name=f"qt{h}")
                for h in range(2)]
        kt_p = [sb.tile([128, SEQ], F16, tag=f"kt{h}", name=f"kt{h}")
                for h in range(2)]
        vones = [sb.tile([128, 4, 130], F16, tag=f"vones{g}", name=f"vones{g}")
                 for g in range(2)]
        # per-head O^T (rows 0:64 data; rows 64:128 zeroed once -- the proj
        # contracts them against wp pads, either side zero suffices but NaN
        # garbage would poison the accumulate)
        otn = [sb.tile([128, SEQ], F16, tag=f"otn{h}", name=f"otn{h}")
               for h in range(2)]
        # 1/Z broadcast rhs: row 0 = reciprocal_approx_fast(Z), rows 1:127
        # zeroed so the 128-contraction ones matmul stays in 128-row mode
        z_sb = [sb.tile([128, SEQ], F32, tag=f"z{h}", name=f"z{h}")
                for h in range(2)]
        rbc_sb = [sb.tile([64, SEQ], F32, tag=f"rbc{h}", name=f"rbc{h}")
                  for h in range(2)]
        ones_pad = sb.tile([128, 64], F32, tag="ones")
        p_t = [sb.tile([128, SEQ], F16, tag=f"pt{i}", name=f"pt{i}")
               for i in range(P_DEPTH)]
        o16_t = [sb.tile([128, C], F16, tag=f"o16{i}", name=f"o16{i}")
                 for i in range(4)]

        # ---- input DMAs: split across both hwdge queues, need-ordered.
        # The ACT queue goes live ~3us before SP, so it carries the front
        # of the stream (wk + first xc/xp chunks). ----
        nc.scalar.dma_start(out=wk_sb[:],
                            in_=wk.rearrange("(ct p) d -> p ct d", p=128))
        nc.scalar.dma_start(out=xc_sb[0][:], in_=xcT[0:128, :])
        nc.scalar.dma_start(out=xc_sb[1][:], in_=xcT[128:256, :])
        nc.scalar.dma_start(out=wq_sb[:],
                            in_=wq.rearrange("(ct p) d -> p ct d", p=128))
        nc.scalar.dma_start(out=xp_sb[0][:], in_=xpT[0:128, :])
        nc.scalar.dma_start(out=xp_sb[1][:], in_=xpT[128:256, :])
        nc.scalar.dma_start(out=wv_sb[:],
                            in_=wv.rearrange("(ct p) d -> p ct d", p=128))
        nc.sync.dma_start(out=xc_sb[2][:], in_=xcT[256:384, :])
        nc.sync.dma_start(out=xc_sb[3][:], in_=xcT[384:512, :])
        nc.sync.dma_start(out=xp_sb[2][:], in_=xpT[256:384, :])
        nc.sync.dma_start(out=xp_sb[3][:], in_=xpT[384:512, :])
        nc.sync.dma_start(out=wp0_sb[0:64, :], in_=wp[0:64, :])
        nc.sync.dma_start(out=wp1_sb[0:64, :], in_=wp[64:128, :])

        # ---- constant / zero-pad memsets on idle engines (Pool + DVE),
        # most-urgent first (kt/qt pads gate the first S^T) ----
        nc.vector.memset(kt_p[0][64:128, :], 0.0)
        nc.vector.memset(qt_p[0][64:128, :], 0.0)
        nc.gpsimd.memset(kt_p[1][0:64, :], 0.0)
        nc.gpsimd.memset(qt_p[1][0:64, :], 0.0)
        nc.gpsimd.memset(z_sb[0][:], 0.0)
        nc.gpsimd.memset(z_sb[1][:], 0.0)
        nc.gpsimd.memset(otn[0][64:128, :], 0.0)
        nc.gpsimd.memset(otn[1][64:128, :], 0.0)
        nc.gpsimd.memset(wp0_sb[64:128, :], 0.0)
        nc.gpsimd.memset(wp1_sb[64:128, :], 0.0)
        nc.vector.memset(ones_pad[:], 0.0)
        nc.vector.memset(ones_pad[0:1, :], 1.0)
        for g in range(2):
            nc.vector.memset(vones[g][:, :, 64:65], 1.0)
            nc.vector.memset(vones[g][:, :, 129:130], 1.0)

        # ---- KT then V then QT on the PE (matches data-arrival order) ----
        with ExitStack() as qctx:
            qkt_pool = qctx.enter_context(
                tc.tile_pool(name="qkt_ps", bufs=1, space="PSUM"))
            v_pool = qctx.enter_context(
                tc.tile_pool(name="v_ps", bufs=1, space="PSUM"))
            kt_ps = [qkt_pool.tile([128, 512], F32, tag=f"ktps{nh}",
                                   name=f"ktps{nh}") for nh in range(2)]
            qt_ps = [qkt_pool.tile([128, 512], F32, tag=f"qtps{nh}",
                                   name=f"qtps{nh}") for nh in range(2)]
            v_ps = [v_pool.tile([128, 128], F32, tag=f"vps{i}",
                                name=f"vps{i}") for i in range(2)]
            for nh in range(2):
                for ct in range(CT_N):
                    nc.tensor.matmul(
                        out=kt_ps[nh][:],
                        lhsT=wk_sb[:, ct, :],
                        rhs=xc_sb[ct][:, nh * 512:(nh + 1) * 512],
                        start=(ct == 0), stop=(ct == CT_N - 1))
            for nh in range(2):
                o = nh * 512
                if nh == 0:
                    nc.vector.tensor_copy(out=kt_p[0][0:64, o:o + 512],
                                          in_=kt_ps[nh][0:64, :])
                    nc.scalar.copy(out=kt_p[1][64:128, o:o + 512],
                                   in_=kt_ps[nh][64:128, :])
                else:
                    nc.scalar.copy(out=kt_p[0][0:64, o:o + 512],
                                   in_=kt_ps[nh][0:64, :])
                    nc.vector.tensor_copy(out=kt_p[1][64:128, o:o + 512],
                                          in_=kt_ps[nh][64:128, :])

            for kt in range(KT_N):
                vt = v_ps[kt % 2]
                for ct in range(CT_N):
                    nc.tensor.matmul(
                        out=vt[:],
                        lhsT=xc_sb[ct][:, kt * 128:(kt + 1) * 128],
                        rhs=wv_sb[:, ct, :],
                        start=(ct == 0), stop=(ct == CT_N - 1))
                dst = vones[kt // 4][:, kt % 4, :].rearrange(
                    "p (g s) -> p g s", g=2)[:, :, 0:64]
                vsrc = vt[:].rearrange("p (g s) -> p g s", g=2)
                nc.vector.tensor_copy(out=dst, in_=vsrc)

            for nh in range(2):
                for ct in range(CT_N):
                    nc.tensor.matmul(
                        out=qt_ps[nh][:],
                        lhsT=wq_sb[:, ct, :],
                        rhs=xp_sb[ct][:, nh * 512:(nh + 1) * 512],
                        start=(ct == 0), stop=(ct == CT_N - 1))
            for nh in range(2):
                o = nh * 512
                if nh == 0:
                    nc.vector.tensor_copy(out=qt_p[0][0:64, o:o + 512],
                                          in_=qt_ps[nh][0:64, :])
                    nc.scalar.copy(out=qt_p[1][64:128, o:o + 512],
                                   in_=qt_ps[nh][64:128, :])
                else:
                    nc.scalar.copy(out=qt_p[0][0:64, o:o + 512],
                                   in_=qt_ps[nh][0:64, :])
                    nc.vector.tensor_copy(out=qt_p[1][64:128, o:o + 512],
                                          in_=qt_ps[nh][64:128, :])

        # ---- attention, head-sequential: S^T -> exp -> PV (+Z via ones
        # column); head h's normalizer tail emitted inline so h0's runs
        # on DVE under h1's exp stream ----
        with ExitStack() as actx:
            pv_pool = actx.enter_context(
                tc.tile_pool(name="pv_ps", bufs=1, space="PSUM"))
            rbc_pool = actx.enter_context(
                tc.tile_pool(name="rbc_ps", bufs=1, space="PSUM"))
            pv = pv_pool.tile([65, SEQ], F32, tag="pv", name="pv")
            rbc_ps = [rbc_pool.tile([64, 512], F32, tag=f"rbcps{i}",
                                    name=f"rbcps{i}") for i in range(2)]
            s_stack = ExitStack()
            s_pool = s_stack.enter_context(
                tc.tile_pool(name="s_ps", bufs=1, space="PSUM"))
            s_t = [s_pool.tile([128, SEQ], F32, tag=f"st{i}", name=f"st{i}")
                   for i in range(2)]
            items = [(h, kt) for h in range(2) for kt in range(KT_N)]

            def emit_st(i):
                h, kt = items[i]
                s = s_t[i % 2]
                for nh in range(2):
                    nc.tensor.matmul(
                        out=s[:, nh * 512:(nh + 1) * 512],
                        lhsT=kt_p[h][:, kt * 128:(kt + 1) * 128],
                        rhs=qt_p[h][:, nh * 512:(nh + 1) * 512],
                        start=True, stop=True)

            def emit_tail(h):
                # 1/Z (18-bit approx, DVE) -> broadcast over 64 d-rows via
                # ones-matmul -> rbc copy to SBUF -> O^T = pv * (1/Z)
                if USE_RECIP_APPROX:
                    nc.vector.reciprocal_approx_fast(
                        out=z_sb[h][0:1, :], in_=pv[64:65, :])
                else:
                    nc.vector.reciprocal(
                        out=z_sb[h][0:1, :], in_=pv[64:65, :])
                for nh in range(2):
                    o = nh * 512
                    nc.tensor.matmul(out=rbc_ps[nh][:], lhsT=ones_pad[:],
                                     rhs=z_sb[h][:, o:o + 512],
                                     start=True, stop=True)
                for nh in range(2):
                    o = nh * 512
                    if h == 0:
                        nc.vector.tensor_copy(out=rbc_sb[h][:, o:o + 512],
                                              in_=rbc_ps[nh][:])
                    else:
                        nc.scalar.copy(out=rbc_sb[h][:, o:o + 512],
                                       in_=rbc_ps[nh][:])
                for nh in range(2):
                    o = nh * 512
                    nc.vector.tensor_mul(out=otn[h][0:64, o:o + 512],
                                         in0=pv[0:64, o:o + 512],
                                         in1=rbc_sb[h][:, o:o + 512])

            emit_st(0)
            for i, (h, kt) in enumerate(items):
                if i + 1 < len(items):
                    emit_st(i + 1)
                p = p_t[i % P_DEPTH]
                nc.scalar.activation(out=p[:], in_=s_t[i % 2][:], func=Exp,
                                     scale=float(SCALE))
                for nh in range(2):
                    nc.tensor.matmul(
                        out=pv[:, nh * 512:(nh + 1) * 512],
                        lhsT=vones[kt // 4][:, kt % 4, h * 65:h * 65 + 65],
                        rhs=p[:, nh * 512:(nh + 1) * 512],
                        start=(kt == 0), stop=(kt == KT_N - 1))
                if kt == KT_N - 1:
                    emit_tail(h)
            s_stack.close()

            # ---- projection, chunked by query tile; out_ps reuses the
            # s_t banks (s_pool closed after the last exp) ----
            with ExitStack() as tctx:
                out_pool = tctx.enter_context(
                    tc.tile_pool(name="out_ps", bufs=1, space="PSUM"))
                out_ps = [out_pool.tile([128, C], F32, tag=f"ops{i}",
                                        name=f"ops{i}") for i in range(2)]
                for qt in range(KT_N):
                    q = qt * 128
                    ot = out_ps[qt % 2]
                    nc.tensor.matmul(out=ot[:],
                                     lhsT=otn[0][:, q:q + 128],
                                     rhs=wp0_sb[:], start=True,
                                     stop=False)
                    nc.tensor.matmul(out=ot[:],
                                     lhsT=otn[1][:, q:q + 128],
                                     rhs=wp1_sb[:], start=False,
                                     stop=True)
                    o16 = o16_t[qt % 4]
                    if qt % 2 == 1:
                        nc.scalar.copy(out=o16[:], in_=ot[:])
                    else:
                        nc.vector.tensor_copy(out=o16[:], in_=ot[:])
                    eng = nc.sync if qt % 2 == 0 else nc.scalar
                    eng.dma_start(
                        out=out[qt * 128:(qt + 1) * 128, :], in_=o16[:])


def _get_program():
    global _PROG
    if _PROG is None:
        _PROG = _build_program()
    return _PROG


def _shard_inputs(x_pred, x_ctx, ctx_mask, Wq, Wkv, Wproj):
    """Build the 8 per-core input maps (host-side sharding)."""
    ctx_mask = np.asarray(ctx_mask).astype(bool)
    pidx = np.nonzero(~ctx_mask.reshape(-1))[0]
    cidx = np.nonzero(ctx_mask.reshape(-1))[0]
    pm = [np.where(pidx // T == b)[0] for b in range(B)]
    cm = [np.where(cidx // T == b)[0] for b in range(B)]
    for b in range(B):
        assert len(pm[b]) == T_CTX and len(cm[b]) == T_CTX, (
            "kernel compiled for T_CTX ctx/pred slots per batch row")

    xpT_b, xcT_b = [], []
    for b in range(B):
        Xp = x_pred[pm[b]].reshape(SEQ, C)
        Xc = x_ctx[cm[b]].reshape(SEQ, C)
        xpT_b.append(np.ascontiguousarray(Xp.T).astype(np.float16))
        xcT_b.append(np.ascontiguousarray(Xc.T).astype(np.float16))

    wq16 = Wq.astype(np.float16)
    wk16 = Wkv[:, :C].astype(np.float16)
    wv16 = Wkv[:, C:].astype(np.float16)
    wp16 = Wproj.astype(np.float16)

    in_maps = []
    for c in range(NCORE):
        b, hp = divmod(c, 4)
        hc = hp * 128
        in_maps.append({
            "xpT": xpT_b[b],
            "xcT": xcT_b[b],
            "wq": np.ascontiguousarray(wq16[:, hc:hc + 128]),
            "wk": np.ascontiguousarray(wk16[:, hc:hc + 128]),
            "wv": np.ascontiguousarray(wv16[:, hc:hc + 128]),
            "wp": np.ascontiguousarray(wp16[hc:hc + 128, :]),
        })
    return in_maps, pm


def _unshard_output(results, pm, bproj, dtype):
    full = np.zeros((B * T_CTX, N, C), dtype)
    for b in range(B):
        acc = results[4 * b]["out"].astype(np.float64)
        for j in range(1, 4):
            acc = acc + results[4 * b + j]["out"]
        acc = (acc + bproj).astype(dtype)
        full[pm[b]] = acc.reshape(T_CTX, N, C)
    return full


def run(inputs, trace=False, **kwargs):
    """Run the SPMD kernel; returns (full_output, BassKernelResults)."""
    from concourse.bass_utils import run_bass_kernel_spmd

    nc = _get_program()
    in_maps, pm = _shard_inputs(inputs["x_pred"], inputs["x_ctx"],
                                inputs["ctx_mask"], inputs["Wq"],
                                inputs["Wkv"], inputs["Wproj"])
    res = run_bass_kernel_spmd(nc, in_maps, list(range(NCORE)), trace=trace,
                               **kwargs)
    out = _unshard_output(res.results, pm, np.asarray(inputs["bproj"]),
                          np.asarray(inputs["x_pred"]).dtype)
    return out, res


def kernel(x_pred, x_ctx, ctx_mask, Wq, Wkv, Wproj, bproj):
    out, _ = run(dict(x_pred=np.asarray(x_pred), x_ctx=np.asarray(x_ctx),
                      ctx_mask=np.asarray(ctx_mask), Wq=np.asarray(Wq),
                      Wkv=np.asarray(Wkv), Wproj=np.asarray(Wproj),
                      bproj=np.asarray(bproj)))
    return out


# revision 19
# speedup vs baseline: 1.0208x; 1.0208x over previous
"""Trainium2 Bass kernel for nn_CrossAttention (packed cross-attention).

Math (verified against the jax reference):
  The reference scatters packed rows into dense slots, runs masked dense
  attention over T*N tokens, and gathers pred rows back.  Because q is zero
  in ctx slots, k/v are zero in pred slots, and (pred x pred) pairs are
  masked to -inf, this is exactly: for each batch b, the packed pred rows
  cross-attend to the packed ctx rows of the same batch:

    Q = Xp_b @ Wq ; [K|V] = Xc_b @ Wkv          (Xp_b, Xc_b: [1024, 512])
    out_b = concat_h( softmax(Q_h K_h^T / 8) V_h ) @ Wproj + bproj

  Softmax needs no max-subtraction: |scores| < ~7 (verified), exp is safe
  in fp32.

Sharding: 8 cores = (2 batches) x (4 head-pairs).  Each core computes two
heads of one batch and the partial output projection for those heads
(row-sharded Wproj); the host sums the 4 partials per batch and adds bproj.

v2 design (v1 measured 58.5us; see git-less changelog in comments):
  - input DMA split across BOTH hwdge queues (ACT queue starts ~2.4us,
    SP ~5.3us) so x slabs land ~2x faster; consumers accumulate in
    arrival order
  - attention runs HEAD-SEQUENTIAL (all kt of h0, then h1) with a single
    shared PV psum tile; head 0's softmax-normalizer tail (reciprocal,
    broadcast, multiply) overlaps head 1's exp stream on the otherwise
    idle DVE, so only head 1's short tail remains at the end
  - 1/Z via DVE reciprocal_approx_fast (18-bit) instead of ACT Ln/Exp:
    keeps the ACT engine 100%-dedicated to the 16-tile exp stream, which
    is the per-core floor (2.1M exps / 128 lanes / 1.2GHz = 13.7us)
  - all constant/zero-pad memsets moved to the idle GpSimd(Pool) engine
  - exp stream software-pipelined one S^T tile ahead (as v1); p_t ring
    deepened to 6 so PV may lag behind exp while pv drains the prev head
  - PSUM: qkv pools close before attention pools open; s_t(4 banks) +
    pv(2) + rbc(2) = 8; out_ps reuses s_t banks after the last exp
"""

import sys

if "/opt/trn_rl_repo" not in sys.path:
    sys.path.insert(0, "/opt/trn_rl_repo")

import numpy as np

B, T, N, C, H = 2, 8, 256, 512, 8
T_CTX = T // 2
HD = C // H            # 64
SEQ = T_CTX * N        # 1024 packed tokens per batch (q and kv)
NCORE = 8
CT_N = C // 128        # 4 contraction tiles over C
KT_N = SEQ // 128      # 8 key tiles
SCALE = HD ** -0.5

_PROG = None
SPLIT_WAITS = True  # walrus needs it; CoreSim chokes on it
USE_RECIP_APPROX = False  # custom-DVE op (one pass vs ~6.4 cyc/elem for exact)


def _build_program():
    import concourse.bass as bass
    import concourse.tile as tile
    from concourse import mybir

    class TrimTailTileContext(tile.TileContext):
        """Skip the second end-of-kernel all-engine barrier: executions of
        the NEFF are serialized by the runtime, and the semaphore clear is
        still ordered after the first barrier on the gpsimd queue."""

        def _drain_and_barrier(self, tick_clock, wait_clock):
            from concourse.vector_clock import ScopedClock

            drain_inst = self.nc.sync.drain()
            wait_clock.add_sem_waits(
                drain_inst.ins, ScopedClock({None: tick_clock.global_clock}))
            self.nc.all_engine_barrier()
            popped = self.nc._tile_sem_poison_stack.pop()
            assert popped is self._sem_poison
            self.nc.clear_and_free_semaphores(
                list(self.sems.allocated().values()))

    F16 = mybir.dt.float16

    nc = bass.Bass("TRN2", target_bir_lowering=False, debug=False,
                   num_devices=NCORE)

    xpT = nc.dram_tensor("xpT", [C, SEQ], F16, kind="ExternalInput").ap()
    xcT = nc.dram_tensor("xcT", [C, SEQ], F16, kind="ExternalInput").ap()
    wq = nc.dram_tensor("wq", [C, 128], F16, kind="ExternalInput").ap()
    wk = nc.dram_tensor("wk", [C, 128], F16, kind="ExternalInput").ap()
    wv = nc.dram_tensor("wv", [C, 128], F16, kind="ExternalInput").ap()
    wp = nc.dram_tensor("wp", [128, C], F16, kind="ExternalInput").ap()
    out = nc.dram_tensor("out", [SEQ, C], F16, kind="ExternalOutput").ap()

    with TrimTailTileContext(nc) as tc:
        _emit(nc, tc, mybir, xpT, xcT, wq, wk, wv, wp, out)
    if SPLIT_WAITS:
        _split_sync_waits(nc, mybir)
    return nc


def _split_sync_waits(nc, mybir):
    """This container's walrus build has tight per-instruction sync-wait
    limits ("Too many sync wait commands": Matmult holds 1 wait command,
    control-class instructions 2).  Tile freely assigns more.  Rewrite each
    block, moving overflow waits onto same-engine NoOps inserted directly
    before the over-limit instruction (safe: the engine queue executes in
    order, so the waits still complete before the instruction runs)."""
    LIMITS = {}
    DEFAULT = 1
    NOP_W = 1
    n = 0
    for fn in nc.m.functions:
        for bb in fn.blocks:
            insts = bb.instructions
            new = []
            changed = False
            for inst in insts:
                si = inst.sync_info
                waits = list(si.on_wait) if si is not None else []
                limit = LIMITS.get(inst.opcode, DEFAULT)
                if len(waits) > limit:
                    extra = waits[:-limit] if limit else waits
                    keep = waits[-limit:] if limit else []
                    # the end-of-kernel drain carries one wait per logical
                    # processor; spread its nops across engines so they
                    # retire in parallel (the following barrier re-syncs),
                    # instead of ~130ns each serially on the sync sequencer
                    if inst.opcode == "Drain" and len(extra) > 4:
                        engs = [mybir.EngineType.SP, mybir.EngineType.PE,
                                mybir.EngineType.DVE,
                                mybir.EngineType.Activation,
                                mybir.EngineType.Pool]
                    else:
                        engs = [inst.engine]
                    for i in range(0, len(extra), NOP_W):
                        nop = mybir.InstNoOp(
                            name=f"I-waitsplit-{n}", ins=[], outs=[],
                            engine=engs[(i // NOP_W) % len(engs)],
                            sync_info=mybir.SyncInfo(
                                on_wait=extra[i:i + NOP_W], on_update=[]))
                        new.append(nop)
                        n += 1
                    inst.sync_info = mybir.SyncInfo(
                        on_wait=keep, on_update=list(si.on_update))
                    changed = True
                new.append(inst)
            if changed:
                bb.instructions = new


def _emit(nc, tc, mybir, xpT, xcT, wq, wk, wv, wp, out):
    from contextlib import ExitStack

    F32 = mybir.dt.float32
    F16 = mybir.dt.float16
    Exp = mybir.ActivationFunctionType.Exp
    Ln = mybir.ActivationFunctionType.Ln

    P_DEPTH = 6

    with ExitStack() as ctx:
        sb = ctx.enter_context(tc.tile_pool(name="sb", bufs=1))

        # separate tiles per DMA chunk / per column half: Tile tracks
        # dependencies at tile granularity, so consumers must not share a
        # tile with unrelated later writes
        xp_sb = [sb.tile([128, SEQ], F16, tag=f"xp{ct}", name=f"xp{ct}")
                 for ct in range(CT_N)]
        xc_sb = [sb.tile([128, SEQ], F16, tag=f"xc{ct}", name=f"xc{ct}")
                 for ct in range(CT_N)]
        wq_sb = sb.tile([128, CT_N, 128], F16, tag="wq")
        wk_sb = sb.tile([128, CT_N, 128], F16, tag="wk")
        wv_sb = sb.tile([128, CT_N, 128], F16, tag="wv")
        wp0_sb = sb.tile([128, C], F16, tag="wp0")
        wp1_sb = sb.tile([128, C], F16, tag="wp1")
        qt_p = [sb.tile([128, SEQ], F16, tag=f"qt{h}", name=f"qt{h}")
                for h in range(2)]
        kt_p = [sb.tile([128, SEQ], F16, tag=f"kt{h}", name=f"kt{h}")
                for h in range(2)]
        vones = [sb.tile([128, 4, 130], F16, tag=f"vones{g}", name=f"vones{g}")
                 for g in range(2)]
        # per-head O^T (rows 0:64 data; rows 64:128 zeroed once -- the proj
        # contracts them against wp pads, either side zero suffices but NaN
        # garbage would poison the accumulate)
        otn = [sb.tile([128, SEQ], F16, tag=f"otn{h}", name=f"otn{h}")
               for h in range(2)]
        # 1/Z broadcast rhs: row 0 = reciprocal_approx_fast(Z), rows 1:127
        # zeroed so the 128-contraction ones matmul stays in 128-row mode
        z_sb = [sb.tile([128, SEQ], F32, tag=f"z{h}", name=f"z{h}")
                for h in range(2)]
        rbc_sb = [sb.tile([64, SEQ], F32, tag=f"rbc{h}", name=f"rbc{h}")
                  for h in range(2)]
        pv_stage = sb.tile([65, SEQ], F32, tag="pvstg", name="pvstg")
        ones_pad = sb.tile([128, 64], F32, tag="ones")
        p_t = [sb.tile([128, SEQ], F16, tag=f"pt{i}", name=f"pt{i}")
               for i in range(P_DEPTH)]
        o16_t = [sb.tile([128, C], F16, tag=f"o16{i}", name=f"o16{i}")
                 for i in range(4)]

        # ---- input DMAs: bulk stream need-ordered on the scalar queue
        # (it goes live ~3us before the SP queue and reaches full HBM rate
        # after a fixed ramp; splitting the stream across queues just pays
        # the ramp twice -- measured).  xp0/xp1 ride the SP queue so the
        # two ramps overlap and the last xp chunk lands sooner. ----
        nc.scalar.dma_start(out=wk_sb[:],
                            in_=wk.rearrange("(ct p) d -> p ct d", p=128))
        for ct in range(CT_N):
            nc.scalar.dma_start(out=xc_sb[ct][:],
                                in_=xcT[ct * 128:(ct + 1) * 128, :])
        nc.scalar.dma_start(out=wv_sb[:],
                            in_=wv.rearrange("(ct p) d -> p ct d", p=128))
        nc.scalar.dma_start(out=wq_sb[:],
                            in_=wq.rearrange("(ct p) d -> p ct d", p=128))
        nc.scalar.dma_start(out=xp_sb[2][:], in_=xpT[256:384, :])
        nc.scalar.dma_start(out=xp_sb[3][:], in_=xpT[384:512, :])
        nc.scalar.dma_start(out=wp0_sb[0:64, :], in_=wp[0:64, :])
        nc.scalar.dma_start(out=wp1_sb[0:64, :], in_=wp[64:128, :])
        nc.sync.dma_start(out=xp_sb[0][:], in_=xpT[0:128, :])
        nc.sync.dma_start(out=xp_sb[1][:], in_=xpT[128:256, :])

        # ---- constant / zero-pad memsets on idle engines (Pool + DVE),
        # most-urgent first (kt/qt pads gate the first S^T) ----
        nc.vector.memset(kt_p[0][64:128, :], 0.0)
        nc.vector.memset(qt_p[0][64:128, :], 0.0)
        nc.gpsimd.memset(kt_p[1][0:64, :], 0.0)
        nc.gpsimd.memset(qt_p[1][0:64, :], 0.0)
        nc.gpsimd.memset(z_sb[0][:], 0.0)
        nc.gpsimd.memset(z_sb[1][:], 0.0)
        nc.gpsimd.memset(otn[0][64:128, :], 0.0)
        nc.gpsimd.memset(otn[1][64:128, :], 0.0)
        nc.gpsimd.memset(wp0_sb[64:128, :], 0.0)
        nc.gpsimd.memset(wp1_sb[64:128, :], 0.0)
        nc.vector.memset(ones_pad[:], 0.0)
        nc.vector.memset(ones_pad[0:1, :], 1.0)
        for g in range(2):
            nc.vector.memset(vones[g][:, :, 64:65], 1.0)
            nc.vector.memset(vones[g][:, :, 129:130], 1.0)

        # ---- KT, V, QT on the PE (matches data-arrival order).  PSUM
        # bank choreography: kt_ps's pool closes before qt_ps opens so QT
        # reuses KT's banks; v_ps holds its own; s_t/pv later grab the
        # freed kt/qt banks + fresh ones and do NOT alias v_ps, so the
        # first S^T needs only the kt/qt evacuations, not V's. ----
        v_stack = ExitStack()
        v_pool = v_stack.enter_context(
            tc.tile_pool(name="v_ps", bufs=1, space="PSUM", side="right"))
        v_ps = [v_pool.tile([128, 128], F32, tag="vps", name="vps")]
        with ExitStack() as kctx:
            kt_pool = kctx.enter_context(
                tc.tile_pool(name="kt_ps", bufs=1, space="PSUM"))
            kt_ps = [kt_pool.tile([128, 512], F32, tag=f"ktps{nh}",
                                  name=f"ktps{nh}") for nh in range(2)]
            for nh in range(2):
                for ct in range(CT_N):
                    nc.tensor.matmul(
                        out=kt_ps[nh][:],
                        lhsT=wk_sb[:, ct, :],
                        rhs=xc_sb[ct][:, nh * 512:(nh + 1) * 512],
                        start=(ct == 0), stop=(ct == CT_N - 1))
            for nh in range(2):
                o = nh * 512
                if nh == 0:
                    nc.vector.tensor_copy(out=kt_p[0][0:64, o:o + 512],
                                          in_=kt_ps[nh][0:64, :])
                    nc.scalar.copy(out=kt_p[1][64:128, o:o + 512],
                                   in_=kt_ps[nh][64:128, :])
                else:
                    nc.scalar.copy(out=kt_p[0][0:64, o:o + 512],
                                   in_=kt_ps[nh][0:64, :])
                    nc.vector.tensor_copy(out=kt_p[1][64:128, o:o + 512],
                                          in_=kt_ps[nh][64:128, :])

        for kt in range(KT_N):
            vt = v_ps[0]
            for ct in range(CT_N):
                nc.tensor.matmul(
                    out=vt[:],
                    lhsT=xc_sb[ct][:, kt * 128:(kt + 1) * 128],
                    rhs=wv_sb[:, ct, :],
                    start=(ct == 0), stop=(ct == CT_N - 1))
            dst = vones[kt // 4][:, kt % 4, :].rearrange(
                "p (g s) -> p g s", g=2)[:, :, 0:64]
            vsrc = vt[:].rearrange("p (g s) -> p g s", g=2)
            nc.vector.tensor_copy(out=dst, in_=vsrc)

        with ExitStack() as qctx:
            qt_pool = qctx.enter_context(
                tc.tile_pool(name="qt_ps", bufs=1, space="PSUM"))
            qt_ps = [qt_pool.tile([128, 512], F32, tag=f"qtps{nh}",
                                  name=f"qtps{nh}") for nh in range(2)]
            for nh in range(2):
                for ct in range(CT_N):
                    nc.tensor.matmul(
                        out=qt_ps[nh][:],
                        lhsT=wq_sb[:, ct, :],
                        rhs=xp_sb[ct][:, nh * 512:(nh + 1) * 512],
                        start=(ct == 0), stop=(ct == CT_N - 1))
            for nh in range(2):
                o = nh * 512
                if nh == 0:
                    nc.vector.tensor_copy(out=qt_p[0][0:64, o:o + 512],
                                          in_=qt_ps[nh][0:64, :])
                    nc.scalar.copy(out=qt_p[1][64:128, o:o + 512],
                                   in_=qt_ps[nh][64:128, :])
                else:
                    nc.scalar.copy(out=qt_p[0][0:64, o:o + 512],
                                   in_=qt_ps[nh][0:64, :])
                    nc.vector.tensor_copy(out=qt_p[1][64:128, o:o + 512],
                                          in_=qt_ps[nh][64:128, :])

        # ---- attention, head-sequential: S^T -> exp -> PV (+Z via ones
        # column).  h0's PV result is staged out of PSUM immediately so
        # h1's PV (in-order PE queue!) only waits ~1 iteration; h0's
        # normalizer then runs from SBUF on the idle DVE under h1's exp
        # stream.  h1's tail reads pv directly (nothing waits on it). ----
        with ExitStack() as actx:
            # right stack: v_ps(1 bank, stays open) + pv(2) + rbc(1);
            # left stack: s_t(4) reusing kt/qt's banks + 2 fresh.
            pv_pool = actx.enter_context(
                tc.tile_pool(name="pv_ps", bufs=1, space="PSUM",
                             side="right"))
            pv = pv_pool.tile([65, SEQ], F32, tag="pv", name="pv")
            rbc_pool = actx.enter_context(
                tc.tile_pool(name="rbc_ps", bufs=1, space="PSUM",
                             side="right"))
            rbc_ps = rbc_pool.tile([64, 512], F32, tag="rbcps",
                                   name="rbcps")
            s_stack = ExitStack()
            s_pool = s_stack.enter_context(
                tc.tile_pool(name="s_ps", bufs=1, space="PSUM"))
            s_t = [s_pool.tile([128, SEQ], F32, tag=f"st{i}", name=f"st{i}")
                   for i in range(2)]
            items = [(h, kt) for h in range(2) for kt in range(KT_N)]

            def emit_st(i):
                h, kt = items[i]
                s = s_t[i % 2]
                for nh in range(2):
                    nc.tensor.matmul(
                        out=s[:, nh * 512:(nh + 1) * 512],
                        lhsT=kt_p[h][:, kt * 128:(kt + 1) * 128],
                        rhs=qt_p[h][:, nh * 512:(nh + 1) * 512],
                        start=True, stop=True)

            def emit_tail(h):
                # normalizer: broadcast 1/Z over the 64 d-rows, then
                # O^T = O_unnorm * (1/Z).  h0 stages pv out of PSUM at
                # once (so h1's PV doesn't block the in-order PE queue)
                # and computes 1/Z with the slow-but-DVE-only reciprocal,
                # all hidden under h1's exp stream.  h1 uses the ACT
                # Ln -> broadcast -> Exp(-x) chain: ACT is free right
                # after the final exp, and the chain is ~3x shorter than
                # DVE reciprocal.
                if h == 0:
                    nc.vector.tensor_copy(out=pv_stage[:], in_=pv[:])
                    src = pv_stage
                    nc.vector.reciprocal(
                        out=z_sb[h][0:1, :], in_=src[64:65, :])
                    for nh in range(2):
                        o = nh * 512
                        nc.tensor.matmul(out=rbc_ps[:], lhsT=ones_pad[:],
                                         rhs=z_sb[h][:, o:o + 512],
                                         start=True, stop=True)
                        nc.vector.tensor_copy(out=rbc_sb[h][:, o:o + 512],
                                              in_=rbc_ps[:])
                else:
                    src = pv
                    for nh in range(2):
                        o = nh * 512
                        nc.scalar.activation(out=z_sb[h][0:1, o:o + 512],
                                             in_=src[64:65, o:o + 512],
                                             func=Ln)
                    for nh in range(2):
                        o = nh * 512
                        nc.tensor.matmul(out=rbc_ps[:], lhsT=ones_pad[:],
                                         rhs=z_sb[h][:, o:o + 512],
                                         start=True, stop=True)
                        nc.scalar.activation(out=rbc_sb[h][:, o:o + 512],
                                             in_=rbc_ps[:], func=Exp,
                                             scale=-1.0)
                for nh in range(2):
                    o = nh * 512
                    nc.vector.tensor_mul(out=otn[h][0:64, o:o + 512],
                                         in0=src[0:64, o:o + 512],
                                         in1=rbc_sb[h][:, o:o + 512])

            emit_st(0)
            for i, (h, kt) in enumerate(items):
                if i + 1 < len(items):
                    emit_st(i + 1)
                p = p_t[i % P_DEPTH]
                nc.scalar.activation(out=p[:], in_=s_t[i % 2][:], func=Exp,
                                     scale=float(SCALE))
                for nh in range(2):
                    nc.tensor.matmul(
                        out=pv[:, nh * 512:(nh + 1) * 512],
                        lhsT=vones[kt // 4][:, kt % 4, h * 65:h * 65 + 65],
                        rhs=p[:, nh * 512:(nh + 1) * 512],
                        start=(kt == 0), stop=(kt == KT_N - 1))
                if kt == KT_N - 1:
                    emit_tail(h)
            s_stack.close()

            # ---- projection, chunked by query tile; out_ps reuses the
            # s_t banks (s_pool closed after the last exp) ----
            with ExitStack() as tctx:
                out_pool = tctx.enter_context(
                    tc.tile_pool(name="out_ps", bufs=1, space="PSUM"))
                out_ps = [out_pool.tile([128, C], F32, tag=f"ops{i}",
                                        name=f"ops{i}") for i in range(2)]
                for qt in range(KT_N):
                    q = qt * 128
                    ot = out_ps[qt % 2]
                    nc.tensor.matmul(out=ot[:],
                                     lhsT=otn[0][:, q:q + 128],
                                     rhs=wp0_sb[:], start=True,
                                     stop=False)
                    nc.tensor.matmul(out=ot[:],
                                     lhsT=otn[1][:, q:q + 128],
                                     rhs=wp1_sb[:], start=False,
                                     stop=True)
                    o16 = o16_t[qt % 4]
                    if qt % 2 == 1:
                        nc.scalar.copy(out=o16[:], in_=ot[:])
                    else:
                        nc.vector.tensor_copy(out=o16[:], in_=ot[:])
                    eng = nc.sync if qt % 2 == 0 else nc.scalar
                    eng.dma_start(
                        out=out[qt * 128:(qt + 1) * 128, :], in_=o16[:])
        v_stack.close()


def _get_program():
    global _PROG
    if _PROG is None:
        _PROG = _build_program()
    return _PROG


def _shard_inputs(x_pred, x_ctx, ctx_mask, Wq, Wkv, Wproj):
    """Build the 8 per-core input maps (host-side sharding)."""
    ctx_mask = np.asarray(ctx_mask).astype(bool)
    pidx = np.nonzero(~ctx_mask.reshape(-1))[0]
    cidx = np.nonzero(ctx_mask.reshape(-1))[0]
    pm = [np.where(pidx // T == b)[0] for b in range(B)]
    cm = [np.where(cidx // T == b)[0] for b in range(B)]
    for b in range(B):
        assert len(pm[b]) == T_CTX and len(cm[b]) == T_CTX, (
            "kernel compiled for T_CTX ctx/pred slots per batch row")

    xpT_b, xcT_b = [], []
    for b in range(B):
        Xp = x_pred[pm[b]].reshape(SEQ, C)
        Xc = x_ctx[cm[b]].reshape(SEQ, C)
        xpT_b.append(np.ascontiguousarray(Xp.T).astype(np.float16))
        xcT_b.append(np.ascontiguousarray(Xc.T).astype(np.float16))

    wq16 = Wq.astype(np.float16)
    wk16 = Wkv[:, :C].astype(np.float16)
    wv16 = Wkv[:, C:].astype(np.float16)
    wp16 = Wproj.astype(np.float16)

    in_maps = []
    for c in range(NCORE):
        b, hp = divmod(c, 4)
        hc = hp * 128
        in_maps.append({
            "xpT": xpT_b[b],
            "xcT": xcT_b[b],
            "wq": np.ascontiguousarray(wq16[:, hc:hc + 128]),
            "wk": np.ascontiguousarray(wk16[:, hc:hc + 128]),
            "wv": np.ascontiguousarray(wv16[:, hc:hc + 128]),
            "wp": np.ascontiguousarray(wp16[hc:hc + 128, :]),
        })
    return in_maps, pm


def _unshard_output(results, pm, bproj, dtype):
    full = np.zeros((B * T_CTX, N, C), dtype)
    for b in range(B):
        acc = results[4 * b]["out"].astype(np.float64)
        for j in range(1, 4):
            acc = acc + results[4 * b + j]["out"]
        acc = (acc + bproj).astype(dtype)
        full[pm[b]] = acc.reshape(T_CTX, N, C)
    return full


def run(inputs, trace=False, **kwargs):
    """Run the SPMD kernel; returns (full_output, BassKernelResults)."""
    from concourse.bass_utils import run_bass_kernel_spmd

    nc = _get_program()
    in_maps, pm = _shard_inputs(inputs["x_pred"], inputs["x_ctx"],
                                inputs["ctx_mask"], inputs["Wq"],
                                inputs["Wkv"], inputs["Wproj"])
    res = run_bass_kernel_spmd(nc, in_maps, list(range(NCORE)), trace=trace,
                               **kwargs)
    out = _unshard_output(res.results, pm, np.asarray(inputs["bproj"]),
                          np.asarray(inputs["x_pred"]).dtype)
    return out, res


def kernel(x_pred, x_ctx, ctx_mask, Wq, Wkv, Wproj, bproj):
    out, _ = run(dict(x_pred=np.asarray(x_pred), x_ctx=np.asarray(x_ctx),
                      ctx_mask=np.asarray(ctx_mask), Wq=np.asarray(Wq),
                      Wkv=np.asarray(Wkv), Wproj=np.asarray(Wproj),
                      bproj=np.asarray(bproj)))
    return out


# revision 25
# speedup vs baseline: 1.1323x; 1.1093x over previous
"""Trainium2 Bass kernel for nn_CrossAttention (packed cross-attention).

Math (verified against the jax reference):
  The reference scatters packed rows into dense slots, runs masked dense
  attention over T*N tokens, and gathers pred rows back.  Because q is zero
  in ctx slots, k/v are zero in pred slots, and (pred x pred) pairs are
  masked to -inf, this is exactly: for each batch b, the packed pred rows
  cross-attend to the packed ctx rows of the same batch:

    Q = Xp_b @ Wq ; [K|V] = Xc_b @ Wkv          (Xp_b, Xc_b: [1024, 512])
    out_b = concat_h( softmax(Q_h K_h^T / 8) V_h ) @ Wproj + bproj

  Softmax needs no max-subtraction: |scores| < ~7 (verified), exp is safe
  in fp32.

Sharding: 8 cores = (2 batches) x (4 head-pairs).  Each core computes two
heads of one batch and the partial output projection for those heads
(row-sharded Wproj); the host sums the 4 partials per batch and adds bproj.

v2 design (v1 measured 58.5us; see git-less changelog in comments):
  - input DMA split across BOTH hwdge queues (ACT queue starts ~2.4us,
    SP ~5.3us) so x slabs land ~2x faster; consumers accumulate in
    arrival order
  - attention runs HEAD-SEQUENTIAL (all kt of h0, then h1) with a single
    shared PV psum tile; head 0's softmax-normalizer tail (reciprocal,
    broadcast, multiply) overlaps head 1's exp stream on the otherwise
    idle DVE, so only head 1's short tail remains at the end
  - 1/Z via DVE reciprocal_approx_fast (18-bit) instead of ACT Ln/Exp:
    keeps the ACT engine 100%-dedicated to the 16-tile exp stream, which
    is the per-core floor (2.1M exps / 128 lanes / 1.2GHz = 13.7us)
  - all constant/zero-pad memsets moved to the idle GpSimd(Pool) engine
  - exp stream software-pipelined one S^T tile ahead (as v1); p_t ring
    deepened to 6 so PV may lag behind exp while pv drains the prev head
  - PSUM: qkv pools close before attention pools open; s_t(4 banks) +
    pv(2) + rbc(2) = 8; out_ps reuses s_t banks after the last exp
"""

import sys

if "/opt/trn_rl_repo" not in sys.path:
    sys.path.insert(0, "/opt/trn_rl_repo")

import numpy as np

B, T, N, C, H = 2, 8, 256, 512, 8
T_CTX = T // 2
HD = C // H            # 64
SEQ = T_CTX * N        # 1024 packed tokens per batch (q and kv)
NCORE = 8
CT_N = C // 128        # 4 contraction tiles over C
KT_N = SEQ // 128      # 8 key tiles
SCALE = HD ** -0.5

_PROG = None
SPLIT_WAITS = True  # walrus needs it; CoreSim chokes on it
USE_RECIP_APPROX = False  # custom-DVE op (one pass vs ~6.4 cyc/elem for exact)


def _build_program():
    import concourse.bass as bass
    import concourse.tile as tile
    from concourse import mybir

    class TrimTailTileContext(tile.TileContext):
        """Skip the second end-of-kernel all-engine barrier: executions of
        the NEFF are serialized by the runtime, and the semaphore clear is
        still ordered after the first barrier on the gpsimd queue."""

        def _drain_and_barrier(self, tick_clock, wait_clock):
            from concourse.vector_clock import ScopedClock

            drain_inst = self.nc.sync.drain()
            wait_clock.add_sem_waits(
                drain_inst.ins, ScopedClock({None: tick_clock.global_clock}))
            self.nc.all_engine_barrier()
            popped = self.nc._tile_sem_poison_stack.pop()
            assert popped is self._sem_poison
            self.nc.clear_and_free_semaphores(
                list(self.sems.allocated().values()))

    F16 = mybir.dt.float16

    nc = bass.Bass("TRN2", target_bir_lowering=False, debug=False,
                   num_devices=NCORE)

    xpT = nc.dram_tensor("xpT", [C, SEQ], F16, kind="ExternalInput").ap()
    xcT = nc.dram_tensor("xcT", [C, SEQ], F16, kind="ExternalInput").ap()
    wq = nc.dram_tensor("wq", [C, 128], F16, kind="ExternalInput").ap()
    wk = nc.dram_tensor("wk", [C, 128], F16, kind="ExternalInput").ap()
    wv = nc.dram_tensor("wv", [C, 128], F16, kind="ExternalInput").ap()
    wp = nc.dram_tensor("wp", [128, C], F16, kind="ExternalInput").ap()
    out = nc.dram_tensor("out", [SEQ, C], F16, kind="ExternalOutput").ap()

    with TrimTailTileContext(nc) as tc:
        _emit(nc, tc, mybir, xpT, xcT, wq, wk, wv, wp, out)
    if SPLIT_WAITS:
        _split_sync_waits(nc, mybir)
    return nc


def _split_sync_waits(nc, mybir):
    """This container's walrus build has tight per-instruction sync-wait
    limits ("Too many sync wait commands": Matmult holds 1 wait command,
    control-class instructions 2).  Tile freely assigns more.  Rewrite each
    block, moving overflow waits onto same-engine NoOps inserted directly
    before the over-limit instruction (safe: the engine queue executes in
    order, so the waits still complete before the instruction runs)."""
    LIMITS = {}
    DEFAULT = 1
    NOP_W = 1
    n = 0
    for fn in nc.m.functions:
        for bb in fn.blocks:
            insts = bb.instructions
            new = []
            changed = False
            for inst in insts:
                si = inst.sync_info
                waits = list(si.on_wait) if si is not None else []
                limit = LIMITS.get(inst.opcode, DEFAULT)
                if len(waits) > limit:
                    extra = waits[:-limit] if limit else waits
                    keep = waits[-limit:] if limit else []
                    # the end-of-kernel drain carries one wait per logical
                    # processor; spread its nops across engines so they
                    # retire in parallel (the following barrier re-syncs),
                    # instead of ~130ns each serially on the sync sequencer
                    if inst.opcode == "Drain" and len(extra) > 4:
                        engs = [mybir.EngineType.SP, mybir.EngineType.PE,
                                mybir.EngineType.DVE,
                                mybir.EngineType.Activation,
                                mybir.EngineType.Pool]
                    else:
                        engs = [inst.engine]
                    for i in range(0, len(extra), NOP_W):
                        nop = mybir.InstNoOp(
                            name=f"I-waitsplit-{n}", ins=[], outs=[],
                            engine=engs[(i // NOP_W) % len(engs)],
                            sync_info=mybir.SyncInfo(
                                on_wait=extra[i:i + NOP_W], on_update=[]))
                        new.append(nop)
                        n += 1
                    inst.sync_info = mybir.SyncInfo(
                        on_wait=keep, on_update=list(si.on_update))
                    changed = True
                new.append(inst)
            if changed:
                bb.instructions = new


def _emit(nc, tc, mybir, xpT, xcT, wq, wk, wv, wp, out):
    from contextlib import ExitStack

    F32 = mybir.dt.float32
    F16 = mybir.dt.float16
    Exp = mybir.ActivationFunctionType.Exp
    Ln = mybir.ActivationFunctionType.Ln

    P_DEPTH = 6

    with ExitStack() as ctx:
        sb = ctx.enter_context(tc.tile_pool(name="sb", bufs=1))

        # separate tiles per DMA chunk / per column half: Tile tracks
        # dependencies at tile granularity, so consumers must not share a
        # tile with unrelated later writes
        xp_sb = [sb.tile([128, SEQ], F16, tag=f"xp{ct}", name=f"xp{ct}")
                 for ct in range(CT_N)]
        xc_sb = [sb.tile([128, SEQ], F16, tag=f"xc{ct}", name=f"xc{ct}")
                 for ct in range(CT_N)]
        wq_sb = sb.tile([128, CT_N, 128], F16, tag="wq")
        wk_sb = sb.tile([128, CT_N, 128], F16, tag="wk")
        wv_sb = sb.tile([128, CT_N, 128], F16, tag="wv")
        wp0_sb = sb.tile([128, C], F16, tag="wp0")
        wp1_sb = sb.tile([128, C], F16, tag="wp1")
        qt_p = [sb.tile([128, SEQ], F16, tag=f"qt{h}", name=f"qt{h}")
                for h in range(2)]
        kt_p = [sb.tile([128, SEQ], F16, tag=f"kt{h}", name=f"kt{h}")
                for h in range(2)]
        vones = [sb.tile([128, 4, 130], F16, tag=f"vones{g}", name=f"vones{g}")
                 for g in range(2)]
        # per-head O^T (rows 0:64 data; rows 64:128 zeroed once -- the proj
        # contracts them against wp pads, either side zero suffices but NaN
        # garbage would poison the accumulate)
        otn = [sb.tile([128, SEQ], F16, tag=f"otn{h}", name=f"otn{h}")
               for h in range(2)]
        # 1/Z broadcast rhs: row 0 = reciprocal_approx_fast(Z), rows 1:127
        # zeroed so the 128-contraction ones matmul stays in 128-row mode
        z_sb = [sb.tile([128, SEQ], F32, tag=f"z{h}", name=f"z{h}")
                for h in range(2)]
        rbc_sb = [sb.tile([64, SEQ], F32, tag=f"rbc{h}", name=f"rbc{h}")
                  for h in range(2)]
        pv_stage = sb.tile([65, SEQ], F32, tag="pvstg", name="pvstg")
        ones_pad = sb.tile([128, 64], F32, tag="ones")
        p_t = [sb.tile([128, SEQ], F16, tag=f"pt{i}", name=f"pt{i}")
               for i in range(P_DEPTH)]
        o16_t = [sb.tile([128, C], F16, tag=f"o16{i}", name=f"o16{i}")
                 for i in range(4)]

        # ---- input DMAs: one strictly-ordered sync-queue stream in
        # need-order (measured ~190GB/s average incl. ramp; the scalar
        # queue only sustains ~85GB/s and splitting pays the ramp twice).
        # The three small tail weights ride the scalar queue. ----
        nc.sync.dma_start(out=wk_sb[:],
                          in_=wk.rearrange("(ct p) d -> p ct d", p=128))
        for ct in range(CT_N):
            nc.sync.dma_start(out=xc_sb[ct][:],
                              in_=xcT[ct * 128:(ct + 1) * 128, :])
        nc.sync.dma_start(out=wq_sb[:],
                          in_=wq.rearrange("(ct p) d -> p ct d", p=128))
        for ct in range(CT_N):
            nc.sync.dma_start(out=xp_sb[ct][:],
                              in_=xpT[ct * 128:(ct + 1) * 128, :])
        nc.scalar.dma_start(out=wv_sb[:],
                            in_=wv.rearrange("(ct p) d -> p ct d", p=128))
        nc.scalar.dma_start(out=wp0_sb[0:64, :], in_=wp[0:64, :])
        nc.scalar.dma_start(out=wp1_sb[0:64, :], in_=wp[64:128, :])

        # ---- constant / zero-pad memsets on idle engines (Pool + DVE),
        # most-urgent first (kt/qt pads gate the first S^T) ----
        nc.vector.memset(kt_p[0][64:128, :], 0.0)
        nc.vector.memset(qt_p[0][64:128, :], 0.0)
        nc.gpsimd.memset(kt_p[1][0:64, :], 0.0)
        nc.gpsimd.memset(qt_p[1][0:64, :], 0.0)
        nc.gpsimd.memset(z_sb[0][:], 0.0)
        nc.gpsimd.memset(z_sb[1][:], 0.0)
        nc.gpsimd.memset(otn[0][64:128, :], 0.0)
        nc.gpsimd.memset(otn[1][64:128, :], 0.0)
        nc.gpsimd.memset(wp0_sb[64:128, :], 0.0)
        nc.gpsimd.memset(wp1_sb[64:128, :], 0.0)
        nc.vector.memset(ones_pad[:], 0.0)
        nc.vector.memset(ones_pad[0:1, :], 1.0)
        for g in range(2):
            nc.vector.memset(vones[g][:, :, 64:65], 1.0)
            nc.vector.memset(vones[g][:, :, 129:130], 1.0)

        # ---- KT, V, QT on the PE (matches data-arrival order).  PSUM
        # bank choreography: kt_ps's pool closes before qt_ps opens so QT
        # reuses KT's banks; v_ps holds its own; s_t/pv later grab the
        # freed kt/qt banks + fresh ones and do NOT alias v_ps, so the
        # first S^T needs only the kt/qt evacuations, not V's. ----
        # right-side pool packing rbc (bank-aligned, first) with the two
        # V ping-pong tiles: 2048+512+512B -> 2 banks
        v_stack = ExitStack()
        v_pool = v_stack.enter_context(
            tc.tile_pool(name="vr_ps", bufs=1, space="PSUM", side="right"))
        rbc_ps = v_pool.tile([64, 512], F32, tag="rbcps", name="rbcps")
        v_ps = [v_pool.tile([128, 128], F32, tag="vps", name="vps")]
        with ExitStack() as kctx:
            kt_pool = kctx.enter_context(
                tc.tile_pool(name="kt_ps", bufs=1, space="PSUM"))
            kt_ps = [kt_pool.tile([128, 512], F32, tag=f"ktps{nh}",
                                  name=f"ktps{nh}") for nh in range(2)]
            for nh in range(2):
                for ct in range(CT_N):
                    nc.tensor.matmul(
                        out=kt_ps[nh][:],
                        lhsT=wk_sb[:, ct, :],
                        rhs=xc_sb[ct][:, nh * 512:(nh + 1) * 512],
                        start=(ct == 0), stop=(ct == CT_N - 1))
            for nh in range(2):
                o = nh * 512
                if nh == 0:
                    nc.vector.tensor_copy(out=kt_p[0][0:64, o:o + 512],
                                          in_=kt_ps[nh][0:64, :])
                    nc.scalar.copy(out=kt_p[1][64:128, o:o + 512],
                                   in_=kt_ps[nh][64:128, :])
                else:
                    nc.scalar.copy(out=kt_p[0][0:64, o:o + 512],
                                   in_=kt_ps[nh][0:64, :])
                    nc.vector.tensor_copy(out=kt_p[1][64:128, o:o + 512],
                                          in_=kt_ps[nh][64:128, :])

        for kt in range(KT_N):
            vt = v_ps[0]
            for ct in range(CT_N):
                nc.tensor.matmul(
                    out=vt[:],
                    lhsT=xc_sb[ct][:, kt * 128:(kt + 1) * 128],
                    rhs=wv_sb[:, ct, :],
                    start=(ct == 0), stop=(ct == CT_N - 1))
            dst = vones[kt // 4][:, kt % 4, :].rearrange(
                "p (g s) -> p g s", g=2)[:, :, 0:64]
            vsrc = vt[:].rearrange("p (g s) -> p g s", g=2)
            # alternate engines so the qt evacuations aren't queued behind
            # all eight vones copies on the DVE
            if kt % 2 == 0:
                nc.vector.tensor_copy(out=dst, in_=vsrc)
            else:
                nc.scalar.copy(out=dst, in_=vsrc)

        with ExitStack() as qctx:
            qt_pool = qctx.enter_context(
                tc.tile_pool(name="qt_ps", bufs=1, space="PSUM"))
            qt_ps = [qt_pool.tile([128, 512], F32, tag=f"qtps{nh}",
                                  name=f"qtps{nh}") for nh in range(2)]
            for nh in range(2):
                for ct in range(CT_N):
                    nc.tensor.matmul(
                        out=qt_ps[nh][:],
                        lhsT=wq_sb[:, ct, :],
                        rhs=xp_sb[ct][:, nh * 512:(nh + 1) * 512],
                        start=(ct == 0), stop=(ct == CT_N - 1))
            for nh in range(2):
                o = nh * 512
                if nh == 0:
                    nc.vector.tensor_copy(out=qt_p[0][0:64, o:o + 512],
                                          in_=qt_ps[nh][0:64, :])
                    nc.scalar.copy(out=qt_p[1][64:128, o:o + 512],
                                   in_=qt_ps[nh][64:128, :])
                else:
                    nc.scalar.copy(out=qt_p[0][0:64, o:o + 512],
                                   in_=qt_ps[nh][0:64, :])
                    nc.vector.tensor_copy(out=qt_p[1][64:128, o:o + 512],
                                          in_=qt_ps[nh][64:128, :])

        # ---- attention, head-sequential: S^T -> exp -> PV (+Z via ones
        # column).  h0's PV result is staged out of PSUM immediately so
        # h1's PV (in-order PE queue!) only waits ~1 iteration; h0's
        # normalizer then runs from SBUF on the idle DVE under h1's exp
        # stream.  h1's tail reads pv directly (nothing waits on it). ----
        with ExitStack() as actx:
            # right stack: vr(2 banks, stays open: rbc + v ping-pong) +
            # pv(2); left stack: s_t(4) reusing kt/qt's banks + 2 fresh.
            pv_pool = actx.enter_context(
                tc.tile_pool(name="pv_ps", bufs=1, space="PSUM",
                             side="right"))
            pv = pv_pool.tile([65, SEQ], F32, tag="pv", name="pv")
            s_stack = ExitStack()
            s_pool = s_stack.enter_context(
                tc.tile_pool(name="s_ps", bufs=1, space="PSUM"))
            s_t = [s_pool.tile([128, SEQ], F32, tag=f"st{i}", name=f"st{i}")
                   for i in range(2)]
            items = [(h, kt) for h in range(2) for kt in range(KT_N)]

            def emit_st(i):
                h, kt = items[i]
                s = s_t[i % 2]
                for nh in range(2):
                    nc.tensor.matmul(
                        out=s[:, nh * 512:(nh + 1) * 512],
                        lhsT=kt_p[h][:, kt * 128:(kt + 1) * 128],
                        rhs=qt_p[h][:, nh * 512:(nh + 1) * 512],
                        start=True, stop=True)

            # h0's normalizer: pv is staged out of PSUM at once (so h1's
            # PV doesn't block the in-order PE queue) and 1/Z computed
            # with the slow-but-DVE-only reciprocal, hidden under h1's
            # exp stream.  Its PE broadcast matmuls are emitted at the
            # very END of the stream: the in-order PE queue must not
            # meet them before the reciprocal has finished.
            def emit_tail0_dve():
                nc.vector.tensor_copy(out=pv_stage[:], in_=pv[:])
                nc.vector.reciprocal(
                    out=z_sb[0][0:1, :], in_=pv_stage[64:65, :])

            def emit_tail0_pe():
                for nh in range(2):
                    o = nh * 512
                    nc.tensor.matmul(out=rbc_ps[:], lhsT=ones_pad[:],
                                     rhs=z_sb[0][:, o:o + 512],
                                     start=True, stop=True)
                    nc.vector.tensor_copy(out=rbc_sb[0][:, o:o + 512],
                                          in_=rbc_ps[:])
                for nh in range(2):
                    o = nh * 512
                    nc.vector.tensor_mul(out=otn[0][0:64, o:o + 512],
                                         in0=pv_stage[0:64, o:o + 512],
                                         in1=rbc_sb[0][:, o:o + 512])

            # h1's tail runs right at stream end on the then-idle ACT:
            # Ln -> broadcast -> Exp(-x) implements the 1/Z reciprocal.
            def emit_tail1():
                for nh in range(2):
                    o = nh * 512
                    nc.scalar.activation(out=z_sb[1][0:1, o:o + 512],
                                         in_=pv[64:65, o:o + 512],
                                         func=Ln)
                for nh in range(2):
                    o = nh * 512
                    nc.tensor.matmul(out=rbc_ps[:], lhsT=ones_pad[:],
                                     rhs=z_sb[1][:, o:o + 512],
                                     start=True, stop=True)
                    nc.scalar.activation(out=rbc_sb[1][:, o:o + 512],
                                         in_=rbc_ps[:], func=Exp,
                                         scale=-1.0)
                for nh in range(2):
                    o = nh * 512
                    nc.vector.tensor_mul(out=otn[1][0:64, o:o + 512],
                                         in0=pv[0:64, o:o + 512],
                                         in1=rbc_sb[1][:, o:o + 512])

            emit_st(0)
            for i, (h, kt) in enumerate(items):
                if i + 1 < len(items):
                    emit_st(i + 1)
                p = p_t[i % P_DEPTH]
                nc.scalar.activation(out=p[:], in_=s_t[i % 2][:], func=Exp,
                                     scale=float(SCALE))
                for nh in range(2):
                    nc.tensor.matmul(
                        out=pv[:, nh * 512:(nh + 1) * 512],
                        lhsT=vones[kt // 4][:, kt % 4, h * 65:h * 65 + 65],
                        rhs=p[:, nh * 512:(nh + 1) * 512],
                        start=(kt == 0), stop=(kt == KT_N - 1))
                if (h, kt) == (0, KT_N - 1):
                    emit_tail0_dve()
                elif (h, kt) == (1, KT_N - 1):
                    emit_tail0_pe()
                    emit_tail1()
            s_stack.close()

            # ---- projection, chunked by query tile; out_ps reuses the
            # s_t banks (s_pool closed after the last exp) ----
            with ExitStack() as tctx:
                out_pool = tctx.enter_context(
                    tc.tile_pool(name="out_ps", bufs=1, space="PSUM"))
                out_ps = [out_pool.tile([128, C], F32, tag=f"ops{i}",
                                        name=f"ops{i}") for i in range(2)]
                for qt in range(KT_N):
                    q = qt * 128
                    ot = out_ps[qt % 2]
                    nc.tensor.matmul(out=ot[:],
                                     lhsT=otn[0][:, q:q + 128],
                                     rhs=wp0_sb[:], start=True,
                                     stop=False)
                    nc.tensor.matmul(out=ot[:],
                                     lhsT=otn[1][:, q:q + 128],
                                     rhs=wp1_sb[:], start=False,
                                     stop=True)
                    o16 = o16_t[qt % 4]
                    if qt % 2 == 1:
                        nc.scalar.copy(out=o16[:], in_=ot[:])
                    else:
                        nc.vector.tensor_copy(out=o16[:], in_=ot[:])
                    eng = nc.sync if qt % 2 == 0 else nc.scalar
                    eng.dma_start(
                        out=out[qt * 128:(qt + 1) * 128, :], in_=o16[:])
        v_stack.close()


def _get_program():
    global _PROG
    if _PROG is None:
        _PROG = _build_program()
    return _PROG


def _shard_inputs(x_pred, x_ctx, ctx_mask, Wq, Wkv, Wproj):
    """Build the 8 per-core input maps (host-side sharding)."""
    ctx_mask = np.asarray(ctx_mask).astype(bool)
    pidx = np.nonzero(~ctx_mask.reshape(-1))[0]
    cidx = np.nonzero(ctx_mask.reshape(-1))[0]
    pm = [np.where(pidx // T == b)[0] for b in range(B)]
    cm = [np.where(cidx // T == b)[0] for b in range(B)]
    for b in range(B):
        assert len(pm[b]) == T_CTX and len(cm[b]) == T_CTX, (
            "kernel compiled for T_CTX ctx/pred slots per batch row")

    xpT_b, xcT_b = [], []
    for b in range(B):
        Xp = x_pred[pm[b]].reshape(SEQ, C)
        Xc = x_ctx[cm[b]].reshape(SEQ, C)
        xpT_b.append(np.ascontiguousarray(Xp.T).astype(np.float16))
        xcT_b.append(np.ascontiguousarray(Xc.T).astype(np.float16))

    wq16 = Wq.astype(np.float16)
    wk16 = Wkv[:, :C].astype(np.float16)
    wv16 = Wkv[:, C:].astype(np.float16)
    wp16 = Wproj.astype(np.float16)

    in_maps = []
    for c in range(NCORE):
        b, hp = divmod(c, 4)
        hc = hp * 128
        in_maps.append({
            "xpT": xpT_b[b],
            "xcT": xcT_b[b],
            "wq": np.ascontiguousarray(wq16[:, hc:hc + 128]),
            "wk": np.ascontiguousarray(wk16[:, hc:hc + 128]),
            "wv": np.ascontiguousarray(wv16[:, hc:hc + 128]),
            "wp": np.ascontiguousarray(wp16[hc:hc + 128, :]),
        })
    return in_maps, pm


def _unshard_output(results, pm, bproj, dtype):
    full = np.zeros((B * T_CTX, N, C), dtype)
    for b in range(B):
        acc = results[4 * b]["out"].astype(np.float64)
        for j in range(1, 4):
            acc = acc + results[4 * b + j]["out"]
        acc = (acc + bproj).astype(dtype)
        full[pm[b]] = acc.reshape(T_CTX, N, C)
    return full


def run(inputs, trace=False, **kwargs):
    """Run the SPMD kernel; returns (full_output, BassKernelResults)."""
    from concourse.bass_utils import run_bass_kernel_spmd

    nc = _get_program()
    in_maps, pm = _shard_inputs(inputs["x_pred"], inputs["x_ctx"],
                                inputs["ctx_mask"], inputs["Wq"],
                                inputs["Wkv"], inputs["Wproj"])
    res = run_bass_kernel_spmd(nc, in_maps, list(range(NCORE)), trace=trace,
                               **kwargs)
    out = _unshard_output(res.results, pm, np.asarray(inputs["bproj"]),
                          np.asarray(inputs["x_pred"]).dtype)
    return out, res


def kernel(x_pred, x_ctx, ctx_mask, Wq, Wkv, Wproj, bproj):
    out, _ = run(dict(x_pred=np.asarray(x_pred), x_ctx=np.asarray(x_ctx),
                      ctx_mask=np.asarray(ctx_mask), Wq=np.asarray(Wq),
                      Wkv=np.asarray(Wkv), Wproj=np.asarray(Wproj),
                      bproj=np.asarray(bproj)))
    return out


# revision 30
# speedup vs baseline: 1.2207x; 1.0781x over previous
"""Trainium2 Bass kernel for nn_CrossAttention (packed cross-attention).

Math (verified against the jax reference):
  The reference scatters packed rows into dense slots, runs masked dense
  attention over T*N tokens, and gathers pred rows back.  Because q is zero
  in ctx slots, k/v are zero in pred slots, and (pred x pred) pairs are
  masked to -inf, this is exactly: for each batch b, the packed pred rows
  cross-attend to the packed ctx rows of the same batch:

    Q = Xp_b @ Wq ; [K|V] = Xc_b @ Wkv          (Xp_b, Xc_b: [1024, 512])
    out_b = concat_h( softmax(Q_h K_h^T / 8) V_h ) @ Wproj + bproj

  Softmax needs no max-subtraction: |scores| < ~7 (verified), exp is safe
  in fp32.

Sharding: 8 cores = (2 batches) x (4 head-pairs).  Each core computes two
heads of one batch and the partial output projection for those heads
(row-sharded Wproj); the host sums the 4 partials per batch and adds bproj.

v2 design (v1 measured 58.5us; see git-less changelog in comments):
  - input DMA split across BOTH hwdge queues (ACT queue starts ~2.4us,
    SP ~5.3us) so x slabs land ~2x faster; consumers accumulate in
    arrival order
  - attention runs HEAD-SEQUENTIAL (all kt of h0, then h1) with a single
    shared PV psum tile; head 0's softmax-normalizer tail (reciprocal,
    broadcast, multiply) overlaps head 1's exp stream on the otherwise
    idle DVE, so only head 1's short tail remains at the end
  - 1/Z via DVE reciprocal_approx_fast (18-bit) instead of ACT Ln/Exp:
    keeps the ACT engine 100%-dedicated to the 16-tile exp stream, which
    is the per-core floor (2.1M exps / 128 lanes / 1.2GHz = 13.7us)
  - all constant/zero-pad memsets moved to the idle GpSimd(Pool) engine
  - exp stream software-pipelined one S^T tile ahead (as v1); p_t ring
    deepened to 6 so PV may lag behind exp while pv drains the prev head
  - PSUM: qkv pools close before attention pools open; s_t(4 banks) +
    pv(2) + rbc(2) = 8; out_ps reuses s_t banks after the last exp
"""

import sys

if "/opt/trn_rl_repo" not in sys.path:
    sys.path.insert(0, "/opt/trn_rl_repo")

import numpy as np

B, T, N, C, H = 2, 8, 256, 512, 8
T_CTX = T // 2
HD = C // H            # 64
SEQ = T_CTX * N        # 1024 packed tokens per batch (q and kv)
NCORE = 8
CT_N = C // 128        # 4 contraction tiles over C
KT_N = SEQ // 128      # 8 key tiles
SCALE = HD ** -0.5

_PROG = None
SPLIT_WAITS = True  # walrus needs it; CoreSim chokes on it
USE_RECIP_APPROX = False  # custom-DVE op (one pass vs ~6.4 cyc/elem for exact)


def _build_program():
    import concourse.bass as bass
    import concourse.tile as tile
    from concourse import mybir

    class TrimTailTileContext(tile.TileContext):
        """Skip the second end-of-kernel all-engine barrier: executions of
        the NEFF are serialized by the runtime, and the semaphore clear is
        still ordered after the first barrier on the gpsimd queue."""

        def _drain_and_barrier(self, tick_clock, wait_clock):
            from concourse.vector_clock import ScopedClock

            drain_inst = self.nc.sync.drain()
            wait_clock.add_sem_waits(
                drain_inst.ins, ScopedClock({None: tick_clock.global_clock}))
            self.nc.all_engine_barrier()
            popped = self.nc._tile_sem_poison_stack.pop()
            assert popped is self._sem_poison
            self.nc.clear_and_free_semaphores(
                list(self.sems.allocated().values()))

    F16 = mybir.dt.float16

    nc = bass.Bass("TRN2", target_bir_lowering=False, debug=False,
                   num_devices=NCORE)

    xpT = nc.dram_tensor("xpT", [C, SEQ], F16, kind="ExternalInput").ap()
    xcT = nc.dram_tensor("xcT", [C, SEQ], F16, kind="ExternalInput").ap()
    wq = nc.dram_tensor("wq", [C, 128], F16, kind="ExternalInput").ap()
    wk = nc.dram_tensor("wk", [C, 128], F16, kind="ExternalInput").ap()
    wv = nc.dram_tensor("wv", [C, 128], F16, kind="ExternalInput").ap()
    wp = nc.dram_tensor("wp", [128, C], F16, kind="ExternalInput").ap()
    out = nc.dram_tensor("out", [SEQ, C], F16, kind="ExternalOutput").ap()

    with TrimTailTileContext(nc) as tc:
        _emit(nc, tc, mybir, xpT, xcT, wq, wk, wv, wp, out)
    if SPLIT_WAITS:
        _split_sync_waits(nc, mybir)
    return nc


def _split_sync_waits(nc, mybir):
    """This container's walrus build has tight per-instruction sync-wait
    limits ("Too many sync wait commands": Matmult holds 1 wait command,
    control-class instructions 2).  Tile freely assigns more.  Rewrite each
    block, moving overflow waits onto same-engine NoOps inserted directly
    before the over-limit instruction (safe: the engine queue executes in
    order, so the waits still complete before the instruction runs)."""
    LIMITS = {}
    DEFAULT = 1
    NOP_W = 1
    n = 0
    for fn in nc.m.functions:
        for bb in fn.blocks:
            insts = bb.instructions
            new = []
            changed = False
            for inst in insts:
                si = inst.sync_info
                waits = list(si.on_wait) if si is not None else []
                limit = LIMITS.get(inst.opcode, DEFAULT)
                if len(waits) > limit:
                    extra = waits[:-limit] if limit else waits
                    keep = waits[-limit:] if limit else []
                    # the end-of-kernel drain carries one wait per logical
                    # processor; spread its nops across engines so they
                    # retire in parallel (the following barrier re-syncs),
                    # instead of ~130ns each serially on the sync sequencer
                    if inst.opcode == "Drain" and len(extra) > 4:
                        engs = [mybir.EngineType.SP, mybir.EngineType.PE,
                                mybir.EngineType.DVE,
                                mybir.EngineType.Activation,
                                mybir.EngineType.Pool]
                    else:
                        engs = [inst.engine]
                    for i in range(0, len(extra), NOP_W):
                        nop = mybir.InstNoOp(
                            name=f"I-waitsplit-{n}", ins=[], outs=[],
                            engine=engs[(i // NOP_W) % len(engs)],
                            sync_info=mybir.SyncInfo(
                                on_wait=extra[i:i + NOP_W], on_update=[]))
                        new.append(nop)
                        n += 1
                    inst.sync_info = mybir.SyncInfo(
                        on_wait=keep, on_update=list(si.on_update))
                    changed = True
                new.append(inst)
            if changed:
                bb.instructions = new


def _emit(nc, tc, mybir, xpT, xcT, wq, wk, wv, wp, out):
    from contextlib import ExitStack

    F32 = mybir.dt.float32
    F16 = mybir.dt.float16
    Exp = mybir.ActivationFunctionType.Exp
    Ln = mybir.ActivationFunctionType.Ln

    P_DEPTH = 6

    with ExitStack() as ctx:
        sb = ctx.enter_context(tc.tile_pool(name="sb", bufs=1))

        # separate tiles per DMA chunk / per column half: Tile tracks
        # dependencies at tile granularity, so consumers must not share a
        # tile with unrelated later writes
        xp_sb = [sb.tile([128, SEQ], F16, tag=f"xp{ct}", name=f"xp{ct}")
                 for ct in range(CT_N)]
        xc_sb = [sb.tile([128, SEQ], F16, tag=f"xc{ct}", name=f"xc{ct}")
                 for ct in range(CT_N)]
        wq_sb = sb.tile([128, CT_N, 128], F16, tag="wq")
        wk_sb = sb.tile([128, CT_N, 128], F16, tag="wk")
        wv_sb = sb.tile([128, CT_N, 128], F16, tag="wv")
        wp0_sb = sb.tile([128, C], F16, tag="wp0")
        wp1_sb = sb.tile([128, C], F16, tag="wp1")
        qt_p = [sb.tile([128, SEQ], F16, tag=f"qt{h}", name=f"qt{h}")
                for h in range(2)]
        kt_p = [sb.tile([128, SEQ], F16, tag=f"kt{h}", name=f"kt{h}")
                for h in range(2)]
        vones = [sb.tile([128, 4, 130], F16, tag=f"vones{g}", name=f"vones{g}")
                 for g in range(2)]
        # per-head O^T (rows 0:64 data; rows 64:128 zeroed once -- the proj
        # contracts them against wp pads, either side zero suffices but NaN
        # garbage would poison the accumulate)
        otn = [sb.tile([128, SEQ], F16, tag=f"otn{h}", name=f"otn{h}")
               for h in range(2)]
        # 1/Z broadcast rhs: row 0 = reciprocal_approx_fast(Z), rows 1:127
        # zeroed so the 128-contraction ones matmul stays in 128-row mode
        z_sb = [sb.tile([128, SEQ], F32, tag=f"z{h}", name=f"z{h}")
                for h in range(2)]
        rbc_sb = [sb.tile([64, SEQ], F32, tag=f"rbc{h}", name=f"rbc{h}")
                  for h in range(2)]
        pv_stage = sb.tile([65, SEQ], F32, tag="pvstg", name="pvstg")
        ones_pad = sb.tile([128, 64], F32, tag="ones")
        p_t = [sb.tile([128, SEQ], F16, tag=f"pt{i}", name=f"pt{i}")
               for i in range(P_DEPTH)]
        o16_t = [sb.tile([128, C], F16, tag=f"o16{i}", name=f"o16{i}")
                 for i in range(4)]

        # ---- input DMAs: two need-ordered streams.  The scalar queue
        # goes live ~3us before the SP queue but only sustains ~half the
        # rate, so it carries the stream HEAD (wk+xc0, which gate the
        # first matmul) and TAIL (xp2/xp3 + small weights), while the SP
        # queue carries the middle.  Consumers accumulate in arrival
        # order. ----
        nc.scalar.dma_start(out=wk_sb[:],
                            in_=wk.rearrange("(ct p) d -> p ct d", p=128))
        nc.scalar.dma_start(out=xc_sb[0][:], in_=xcT[0:128, :])
        nc.scalar.dma_start(out=xp_sb[2][:], in_=xpT[256:384, :])
        nc.scalar.dma_start(out=xp_sb[3][:], in_=xpT[384:512, :])
        nc.scalar.dma_start(out=wv_sb[:],
                            in_=wv.rearrange("(ct p) d -> p ct d", p=128))
        nc.scalar.dma_start(out=wp0_sb[0:64, :], in_=wp[0:64, :])
        nc.scalar.dma_start(out=wp1_sb[0:64, :], in_=wp[64:128, :])
        for ct in range(1, CT_N):
            nc.sync.dma_start(out=xc_sb[ct][:],
                              in_=xcT[ct * 128:(ct + 1) * 128, :])
        nc.sync.dma_start(out=wq_sb[:],
                          in_=wq.rearrange("(ct p) d -> p ct d", p=128))
        nc.sync.dma_start(out=xp_sb[0][:], in_=xpT[0:128, :])
        nc.sync.dma_start(out=xp_sb[1][:], in_=xpT[128:256, :])

        # ---- constant / zero-pad memsets on idle engines (Pool + DVE),
        # most-urgent first (kt/qt pads gate the first S^T) ----
        nc.vector.memset(kt_p[0][64:128, :], 0.0)
        nc.vector.memset(qt_p[0][64:128, :], 0.0)
        nc.gpsimd.memset(kt_p[1][0:64, :], 0.0)
        nc.gpsimd.memset(qt_p[1][0:64, :], 0.0)
        nc.gpsimd.memset(z_sb[0][:], 0.0)
        nc.gpsimd.memset(z_sb[1][:], 0.0)
        nc.gpsimd.memset(otn[0][64:128, :], 0.0)
        nc.gpsimd.memset(otn[1][64:128, :], 0.0)
        nc.gpsimd.memset(wp0_sb[64:128, :], 0.0)
        nc.gpsimd.memset(wp1_sb[64:128, :], 0.0)
        nc.vector.memset(ones_pad[:], 0.0)
        nc.vector.memset(ones_pad[0:1, :], 1.0)
        for g in range(2):
            nc.vector.memset(vones[g][:, :, 64:65], 1.0)
            nc.vector.memset(vones[g][:, :, 129:130], 1.0)

        # ---- KT, V, QT on the PE (matches data-arrival order).  PSUM
        # bank choreography: kt_ps's pool closes before qt_ps opens so QT
        # reuses KT's banks; v_ps holds its own; s_t/pv later grab the
        # freed kt/qt banks + fresh ones and do NOT alias v_ps, so the
        # first S^T needs only the kt/qt evacuations, not V's. ----
        # right-side pool packing rbc (bank-aligned, first) with the two
        # V ping-pong tiles: 2048+512+512B -> 2 banks
        v_stack = ExitStack()
        v_pool = v_stack.enter_context(
            tc.tile_pool(name="vr_ps", bufs=1, space="PSUM", side="right"))
        rbc_ps = v_pool.tile([64, 512], F32, tag="rbcps", name="rbcps")
        v_ps = [v_pool.tile([128, 128], F32, tag="vps", name="vps")]
        with ExitStack() as kctx:
            kt_pool = kctx.enter_context(
                tc.tile_pool(name="kt_ps", bufs=1, space="PSUM"))
            kt_ps = [kt_pool.tile([128, 512], F32, tag=f"ktps{nh}",
                                  name=f"ktps{nh}") for nh in range(2)]
            for nh in range(2):
                for ct in range(CT_N):
                    nc.tensor.matmul(
                        out=kt_ps[nh][:],
                        lhsT=wk_sb[:, ct, :],
                        rhs=xc_sb[ct][:, nh * 512:(nh + 1) * 512],
                        start=(ct == 0), stop=(ct == CT_N - 1))
            for nh in range(2):
                o = nh * 512
                if nh == 0:
                    nc.vector.tensor_copy(out=kt_p[0][0:64, o:o + 512],
                                          in_=kt_ps[nh][0:64, :])
                    nc.scalar.copy(out=kt_p[1][64:128, o:o + 512],
                                   in_=kt_ps[nh][64:128, :])
                else:
                    nc.scalar.copy(out=kt_p[0][0:64, o:o + 512],
                                   in_=kt_ps[nh][0:64, :])
                    nc.vector.tensor_copy(out=kt_p[1][64:128, o:o + 512],
                                          in_=kt_ps[nh][64:128, :])

        for kt in range(KT_N):
            vt = v_ps[0]
            for ct in range(CT_N):
                nc.tensor.matmul(
                    out=vt[:],
                    lhsT=xc_sb[ct][:, kt * 128:(kt + 1) * 128],
                    rhs=wv_sb[:, ct, :],
                    start=(ct == 0), stop=(ct == CT_N - 1))
            dst = vones[kt // 4][:, kt % 4, :].rearrange(
                "p (g s) -> p g s", g=2)[:, :, 0:64]
            vsrc = vt[:].rearrange("p (g s) -> p g s", g=2)
            # alternate engines so the qt evacuations aren't queued behind
            # all eight vones copies on the DVE
            if kt % 2 == 0:
                nc.vector.tensor_copy(out=dst, in_=vsrc)
            else:
                nc.scalar.copy(out=dst, in_=vsrc)

        with ExitStack() as qctx:
            qt_pool = qctx.enter_context(
                tc.tile_pool(name="qt_ps", bufs=1, space="PSUM"))
            qt_ps = [qt_pool.tile([128, 512], F32, tag=f"qtps{nh}",
                                  name=f"qtps{nh}") for nh in range(2)]
            # ct order matches xp chunk arrival (xp2/xp3 ride the early
            # scalar queue, xp0/xp1 the sync queue)
            qt_order = [2, 3, 0, 1]
            for nh in range(2):
                for j, ct in enumerate(qt_order):
                    nc.tensor.matmul(
                        out=qt_ps[nh][:],
                        lhsT=wq_sb[:, ct, :],
                        rhs=xp_sb[ct][:, nh * 512:(nh + 1) * 512],
                        start=(j == 0), stop=(j == CT_N - 1))
            for nh in range(2):
                o = nh * 512
                if nh == 0:
                    nc.vector.tensor_copy(out=qt_p[0][0:64, o:o + 512],
                                          in_=qt_ps[nh][0:64, :])
                    nc.scalar.copy(out=qt_p[1][64:128, o:o + 512],
                                   in_=qt_ps[nh][64:128, :])
                else:
                    nc.scalar.copy(out=qt_p[0][0:64, o:o + 512],
                                   in_=qt_ps[nh][0:64, :])
                    nc.vector.tensor_copy(out=qt_p[1][64:128, o:o + 512],
                                          in_=qt_ps[nh][64:128, :])

        # ---- attention, head-sequential: S^T -> exp -> PV (+Z via ones
        # column).  h0's PV result is staged out of PSUM immediately so
        # h1's PV (in-order PE queue!) only waits ~1 iteration; h0's
        # normalizer then runs from SBUF on the idle DVE under h1's exp
        # stream.  h1's tail reads pv directly (nothing waits on it). ----
        with ExitStack() as actx:
            # right stack: vr(2 banks, stays open: rbc + v ping-pong) +
            # pv(2); left stack: s_t(4) reusing kt/qt's banks + 2 fresh.
            pv_pool = actx.enter_context(
                tc.tile_pool(name="pv_ps", bufs=1, space="PSUM",
                             side="right"))
            pv = pv_pool.tile([65, SEQ], F32, tag="pv", name="pv")
            s_stack = ExitStack()
            s_pool = s_stack.enter_context(
                tc.tile_pool(name="s_ps", bufs=1, space="PSUM"))
            s_t = [s_pool.tile([128, SEQ], F32, tag=f"st{i}", name=f"st{i}")
                   for i in range(2)]
            items = [(h, kt) for h in range(2) for kt in range(KT_N)]

            def emit_st(i):
                h, kt = items[i]
                s = s_t[i % 2]
                for nh in range(2):
                    nc.tensor.matmul(
                        out=s[:, nh * 512:(nh + 1) * 512],
                        lhsT=kt_p[h][:, kt * 128:(kt + 1) * 128],
                        rhs=qt_p[h][:, nh * 512:(nh + 1) * 512],
                        start=True, stop=True)

            # Normalizer tails.  Both heads use the ACT Ln -> broadcast ->
            # Exp(-x) chain (the DVE reciprocal is ~6.4 cyc/elem AND the
            # tile scheduler mis-models it as ~1us, wrecking the PE
            # order).  h0's chain is dribbled into the exp stream one op
            # per iteration (costs ~2.4us of ACT there, but no stalls);
            # h1's runs at stream end when ACT goes idle.  h0's pv is
            # staged to SBUF first so h1's PV accumulation (in-order PE
            # queue) isn't blocked behind h0's consumers.
            def tail0_step(step):
                if step == 0:
                    nc.vector.tensor_copy(out=pv_stage[:], in_=pv[:])
                elif step == 1:
                    nc.scalar.activation(out=z_sb[0][0:1, :],
                                         in_=pv_stage[64:65, :], func=Ln)
                elif step in (2, 3):
                    o = (step - 2) * 512
                    nc.tensor.matmul(out=rbc_ps[:], lhsT=ones_pad[:],
                                     rhs=z_sb[0][:, o:o + 512],
                                     start=True, stop=True)
                    nc.scalar.activation(out=rbc_sb[0][:, o:o + 512],
                                         in_=rbc_ps[:], func=Exp,
                                         scale=-1.0)
                elif step == 4:
                    for nh in range(2):
                        o = nh * 512
                        nc.vector.tensor_mul(out=otn[0][0:64, o:o + 512],
                                             in0=pv_stage[0:64, o:o + 512],
                                             in1=rbc_sb[0][:, o:o + 512])

            def emit_tail1():
                for nh in range(2):
                    o = nh * 512
                    nc.scalar.activation(out=z_sb[1][0:1, o:o + 512],
                                         in_=pv[64:65, o:o + 512],
                                         func=Ln)
                for nh in range(2):
                    o = nh * 512
                    nc.tensor.matmul(out=rbc_ps[:], lhsT=ones_pad[:],
                                     rhs=z_sb[1][:, o:o + 512],
                                     start=True, stop=True)
                    nc.scalar.activation(out=rbc_sb[1][:, o:o + 512],
                                         in_=rbc_ps[:], func=Exp,
                                         scale=-1.0)
                for nh in range(2):
                    o = nh * 512
                    nc.vector.tensor_mul(out=otn[1][0:64, o:o + 512],
                                         in0=pv[0:64, o:o + 512],
                                         in1=rbc_sb[1][:, o:o + 512])

            emit_st(0)
            for i, (h, kt) in enumerate(items):
                if i + 1 < len(items):
                    emit_st(i + 1)
                p = p_t[i % P_DEPTH]
                nc.scalar.activation(out=p[:], in_=s_t[i % 2][:], func=Exp,
                                     scale=float(SCALE))
                for nh in range(2):
                    nc.tensor.matmul(
                        out=pv[:, nh * 512:(nh + 1) * 512],
                        lhsT=vones[kt // 4][:, kt % 4, h * 65:h * 65 + 65],
                        rhs=p[:, nh * 512:(nh + 1) * 512],
                        start=(kt == 0), stop=(kt == KT_N - 1))
                if 7 <= i <= 11:
                    tail0_step(i - 7)
                if i == 15:
                    emit_tail1()
            s_stack.close()

            # ---- projection, chunked by query tile; out_ps reuses the
            # s_t banks (s_pool closed after the last exp) ----
            with ExitStack() as tctx:
                out_pool = tctx.enter_context(
                    tc.tile_pool(name="out_ps", bufs=1, space="PSUM"))
                out_ps = [out_pool.tile([128, C], F32, tag=f"ops{i}",
                                        name=f"ops{i}") for i in range(2)]
                for qt in range(KT_N):
                    q = qt * 128
                    ot = out_ps[qt % 2]
                    nc.tensor.matmul(out=ot[:],
                                     lhsT=otn[0][:, q:q + 128],
                                     rhs=wp0_sb[:], start=True,
                                     stop=False)
                    nc.tensor.matmul(out=ot[:],
                                     lhsT=otn[1][:, q:q + 128],
                                     rhs=wp1_sb[:], start=False,
                                     stop=True)
                    # alternate evac engines so the proj pipeline is
                    # PE-paced, not evacuation-paced (Pool can't read PSUM)
                    o16 = o16_t[qt % 4]
                    if qt % 2 == 0:
                        nc.vector.tensor_copy(out=o16[:], in_=ot[:])
                    else:
                        nc.scalar.copy(out=o16[:], in_=ot[:])
                    eng = nc.sync if qt % 2 == 0 else nc.scalar
                    eng.dma_start(
                        out=out[qt * 128:(qt + 1) * 128, :], in_=o16[:])
        v_stack.close()


def _get_program():
    global _PROG
    if _PROG is None:
        _PROG = _build_program()
    return _PROG


def _shard_inputs(x_pred, x_ctx, ctx_mask, Wq, Wkv, Wproj):
    """Build the 8 per-core input maps (host-side sharding)."""
    ctx_mask = np.asarray(ctx_mask).astype(bool)
    pidx = np.nonzero(~ctx_mask.reshape(-1))[0]
    cidx = np.nonzero(ctx_mask.reshape(-1))[0]
    pm = [np.where(pidx // T == b)[0] for b in range(B)]
    cm = [np.where(cidx // T == b)[0] for b in range(B)]
    for b in range(B):
        assert len(pm[b]) == T_CTX and len(cm[b]) == T_CTX, (
            "kernel compiled for T_CTX ctx/pred slots per batch row")

    xpT_b, xcT_b = [], []
    for b in range(B):
        Xp = x_pred[pm[b]].reshape(SEQ, C)
        Xc = x_ctx[cm[b]].reshape(SEQ, C)
        xpT_b.append(np.ascontiguousarray(Xp.T).astype(np.float16))
        xcT_b.append(np.ascontiguousarray(Xc.T).astype(np.float16))

    wq16 = Wq.astype(np.float16)
    wk16 = Wkv[:, :C].astype(np.float16)
    wv16 = Wkv[:, C:].astype(np.float16)
    wp16 = Wproj.astype(np.float16)

    in_maps = []
    for c in range(NCORE):
        b, hp = divmod(c, 4)
        hc = hp * 128
        in_maps.append({
            "xpT": xpT_b[b],
            "xcT": xcT_b[b],
            "wq": np.ascontiguousarray(wq16[:, hc:hc + 128]),
            "wk": np.ascontiguousarray(wk16[:, hc:hc + 128]),
            "wv": np.ascontiguousarray(wv16[:, hc:hc + 128]),
            "wp": np.ascontiguousarray(wp16[hc:hc + 128, :]),
        })
    return in_maps, pm


def _unshard_output(results, pm, bproj, dtype):
    full = np.zeros((B * T_CTX, N, C), dtype)
    for b in range(B):
        acc = results[4 * b]["out"].astype(np.float64)
        for j in range(1, 4):
            acc = acc + results[4 * b + j]["out"]
        acc = (acc + bproj).astype(dtype)
        full[pm[b]] = acc.reshape(T_CTX, N, C)
    return full


def run(inputs, trace=False, **kwargs):
    """Run the SPMD kernel; returns (full_output, BassKernelResults)."""
    from concourse.bass_utils import run_bass_kernel_spmd

    nc = _get_program()
    in_maps, pm = _shard_inputs(inputs["x_pred"], inputs["x_ctx"],
                                inputs["ctx_mask"], inputs["Wq"],
                                inputs["Wkv"], inputs["Wproj"])
    res = run_bass_kernel_spmd(nc, in_maps, list(range(NCORE)), trace=trace,
                               **kwargs)
    out = _unshard_output(res.results, pm, np.asarray(inputs["bproj"]),
                          np.asarray(inputs["x_pred"]).dtype)
    return out, res


def kernel(x_pred, x_ctx, ctx_mask, Wq, Wkv, Wproj, bproj):
    out, _ = run(dict(x_pred=np.asarray(x_pred), x_ctx=np.asarray(x_ctx),
                      ctx_mask=np.asarray(ctx_mask), Wq=np.asarray(Wq),
                      Wkv=np.asarray(Wkv), Wproj=np.asarray(Wproj),
                      bproj=np.asarray(bproj)))
    return out


# revision 34
# speedup vs baseline: 1.2569x; 1.0297x over previous
"""Trainium2 Bass kernel for nn_CrossAttention (packed cross-attention).

Math (verified against the jax reference):
  The reference scatters packed rows into dense slots, runs masked dense
  attention over T*N tokens, and gathers pred rows back.  Because q is zero
  in ctx slots, k/v are zero in pred slots, and (pred x pred) pairs are
  masked to -inf, this is exactly: for each batch b, the packed pred rows
  cross-attend to the packed ctx rows of the same batch:

    Q = Xp_b @ Wq ; [K|V] = Xc_b @ Wkv          (Xp_b, Xc_b: [1024, 512])
    out_b = concat_h( softmax(Q_h K_h^T / 8) V_h ) @ Wproj + bproj

  Softmax needs no max-subtraction: |scores| < ~7 (verified), exp is safe
  in fp32.

Sharding: 8 cores = (2 batches) x (4 head-pairs).  Each core computes two
heads of one batch and the partial output projection for those heads
(row-sharded Wproj); the host sums the 4 partials per batch and adds bproj.

v2 design (v1 measured 58.5us; see git-less changelog in comments):
  - input DMA split across BOTH hwdge queues (ACT queue starts ~2.4us,
    SP ~5.3us) so x slabs land ~2x faster; consumers accumulate in
    arrival order
  - attention runs HEAD-SEQUENTIAL (all kt of h0, then h1) with a single
    shared PV psum tile; head 0's softmax-normalizer tail (reciprocal,
    broadcast, multiply) overlaps head 1's exp stream on the otherwise
    idle DVE, so only head 1's short tail remains at the end
  - 1/Z via DVE reciprocal_approx_fast (18-bit) instead of ACT Ln/Exp:
    keeps the ACT engine 100%-dedicated to the 16-tile exp stream, which
    is the per-core floor (2.1M exps / 128 lanes / 1.2GHz = 13.7us)
  - all constant/zero-pad memsets moved to the idle GpSimd(Pool) engine
  - exp stream software-pipelined one S^T tile ahead (as v1); p_t ring
    deepened to 6 so PV may lag behind exp while pv drains the prev head
  - PSUM: qkv pools close before attention pools open; s_t(4 banks) +
    pv(2) + rbc(2) = 8; out_ps reuses s_t banks after the last exp
"""

import sys

if "/opt/trn_rl_repo" not in sys.path:
    sys.path.insert(0, "/opt/trn_rl_repo")

import numpy as np

B, T, N, C, H = 2, 8, 256, 512, 8
T_CTX = T // 2
HD = C // H            # 64
SEQ = T_CTX * N        # 1024 packed tokens per batch (q and kv)
NCORE = 8
CT_N = C // 128        # 4 contraction tiles over C
KT_N = SEQ // 128      # 8 key tiles
SCALE = HD ** -0.5

_PROG = None
SPLIT_WAITS = True  # walrus needs it; CoreSim chokes on it
USE_RECIP_APPROX = False  # custom-DVE op (one pass vs ~6.4 cyc/elem for exact)


def _build_program():
    import concourse.bass as bass
    import concourse.tile as tile
    from concourse import mybir

    class TrimTailTileContext(tile.TileContext):
        """Skip the second end-of-kernel all-engine barrier: executions of
        the NEFF are serialized by the runtime, and the semaphore clear is
        still ordered after the first barrier on the gpsimd queue."""

        def _drain_and_barrier(self, tick_clock, wait_clock):
            from concourse.vector_clock import ScopedClock

            drain_inst = self.nc.sync.drain()
            wait_clock.add_sem_waits(
                drain_inst.ins, ScopedClock({None: tick_clock.global_clock}))
            self.nc.all_engine_barrier()
            popped = self.nc._tile_sem_poison_stack.pop()
            assert popped is self._sem_poison
            self.nc.clear_and_free_semaphores(
                list(self.sems.allocated().values()))

    F16 = mybir.dt.float16

    nc = bass.Bass("TRN2", target_bir_lowering=False, debug=False,
                   num_devices=NCORE)

    xpT = nc.dram_tensor("xpT", [C, SEQ], F16, kind="ExternalInput").ap()
    xcT = nc.dram_tensor("xcT", [C, SEQ], F16, kind="ExternalInput").ap()
    wq = nc.dram_tensor("wq", [C, 128], F16, kind="ExternalInput").ap()
    wk = nc.dram_tensor("wk", [C, 128], F16, kind="ExternalInput").ap()
    wv = nc.dram_tensor("wv", [C, 128], F16, kind="ExternalInput").ap()
    wp = nc.dram_tensor("wp", [128, C], F16, kind="ExternalInput").ap()
    out = nc.dram_tensor("out", [SEQ, C], F16, kind="ExternalOutput").ap()

    with TrimTailTileContext(nc) as tc:
        _emit(nc, tc, mybir, xpT, xcT, wq, wk, wv, wp, out)
    if SPLIT_WAITS:
        _split_sync_waits(nc, mybir)
    return nc


def _split_sync_waits(nc, mybir):
    """This container's walrus build has tight per-instruction sync-wait
    limits ("Too many sync wait commands": Matmult holds 1 wait command,
    control-class instructions 2).  Tile freely assigns more.  Rewrite each
    block, moving overflow waits onto same-engine NoOps inserted directly
    before the over-limit instruction (safe: the engine queue executes in
    order, so the waits still complete before the instruction runs)."""
    LIMITS = {}
    DEFAULT = 1
    NOP_W = 1
    n = 0
    for fn in nc.m.functions:
        for bb in fn.blocks:
            insts = bb.instructions
            new = []
            changed = False
            for inst in insts:
                si = inst.sync_info
                waits = list(si.on_wait) if si is not None else []
                limit = LIMITS.get(inst.opcode, DEFAULT)
                if len(waits) > limit:
                    extra = waits[:-limit] if limit else waits
                    keep = waits[-limit:] if limit else []
                    # the end-of-kernel drain carries one wait per logical
                    # processor; spread its nops across engines so they
                    # retire in parallel (the following barrier re-syncs),
                    # instead of ~130ns each serially on the sync sequencer
                    if inst.opcode == "Drain" and len(extra) > 4:
                        engs = [mybir.EngineType.SP, mybir.EngineType.PE,
                                mybir.EngineType.DVE,
                                mybir.EngineType.Activation,
                                mybir.EngineType.Pool]
                    else:
                        engs = [inst.engine]
                    for i in range(0, len(extra), NOP_W):
                        nop = mybir.InstNoOp(
                            name=f"I-waitsplit-{n}", ins=[], outs=[],
                            engine=engs[(i // NOP_W) % len(engs)],
                            sync_info=mybir.SyncInfo(
                                on_wait=extra[i:i + NOP_W], on_update=[]))
                        new.append(nop)
                        n += 1
                    inst.sync_info = mybir.SyncInfo(
                        on_wait=keep, on_update=list(si.on_update))
                    changed = True
                new.append(inst)
            if changed:
                bb.instructions = new


def _emit(nc, tc, mybir, xpT, xcT, wq, wk, wv, wp, out):
    from contextlib import ExitStack

    F32 = mybir.dt.float32
    F16 = mybir.dt.float16
    Exp = mybir.ActivationFunctionType.Exp
    Ln = mybir.ActivationFunctionType.Ln

    P_DEPTH = 6

    with ExitStack() as ctx:
        sb = ctx.enter_context(tc.tile_pool(name="sb", bufs=1))

        # separate tiles per DMA chunk / per column half: Tile tracks
        # dependencies at tile granularity, so consumers must not share a
        # tile with unrelated later writes
        xp_sb = [sb.tile([128, SEQ], F16, tag=f"xp{ct}", name=f"xp{ct}")
                 for ct in range(CT_N)]
        xc_sb = [sb.tile([128, SEQ], F16, tag=f"xc{ct}", name=f"xc{ct}")
                 for ct in range(CT_N)]
        wq_sb = sb.tile([128, CT_N, 128], F16, tag="wq")
        wk_sb = sb.tile([128, CT_N, 128], F16, tag="wk")
        wv_sb = sb.tile([128, CT_N, 128], F16, tag="wv")
        wp0_sb = sb.tile([128, C], F16, tag="wp0")
        wp1_sb = sb.tile([128, C], F16, tag="wp1")
        qt_p = [sb.tile([128, SEQ], F16, tag=f"qt{h}", name=f"qt{h}")
                for h in range(2)]
        kt_p = [sb.tile([128, SEQ], F16, tag=f"kt{h}", name=f"kt{h}")
                for h in range(2)]
        vones = [sb.tile([128, 4, 130], F16, tag=f"vones{g}", name=f"vones{g}")
                 for g in range(2)]
        # per-head O^T (rows 0:64 data; rows 64:128 zeroed once -- the proj
        # contracts them against wp pads, either side zero suffices but NaN
        # garbage would poison the accumulate)
        otn = [sb.tile([128, SEQ], F16, tag=f"otn{h}", name=f"otn{h}")
               for h in range(2)]
        # 1/Z broadcast rhs: row 0 = reciprocal_approx_fast(Z), rows 1:127
        # zeroed so the 128-contraction ones matmul stays in 128-row mode
        z_sb = [sb.tile([128, SEQ], F32, tag=f"z{h}", name=f"z{h}")
                for h in range(2)]
        rbc_sb = [sb.tile([64, SEQ], F32, tag=f"rbc{h}", name=f"rbc{h}")
                  for h in range(2)]
        pv_stage = sb.tile([65, SEQ], F32, tag="pvstg", name="pvstg")
        ones_pad = sb.tile([128, 64], F32, tag="ones")
        p_t = [sb.tile([128, SEQ], F16, tag=f"pt{i}", name=f"pt{i}")
               for i in range(P_DEPTH)]
        o16_t = [sb.tile([128, C], F16, tag=f"o16{i}", name=f"o16{i}")
                 for i in range(4)]

        # ---- input DMAs.  The SP queue is the only fast one (~190GB/s
        # avg; it starts executing ~8.6us in) -- it carries all the bulk,
        # xc first.  The scalar queue starts ~2.4us but each dma config
        # burns ~2us of ACT sequencer, so it gets exactly the transfers
        # that pay off early: wk (gates the first matmul, lands ~4us) and
        # xp3 (gives QT a head start), plus the small late weights. ----
        nc.scalar.dma_start(out=wk_sb[:],
                            in_=wk.rearrange("(ct p) d -> p ct d", p=128))
        nc.scalar.dma_start(out=xp_sb[3][:], in_=xpT[384:512, :])
        nc.scalar.dma_start(out=wv_sb[:],
                            in_=wv.rearrange("(ct p) d -> p ct d", p=128))
        nc.scalar.dma_start(out=wp0_sb[0:64, :], in_=wp[0:64, :])
        nc.scalar.dma_start(out=wp1_sb[0:64, :], in_=wp[64:128, :])
        nc.sync.dma_start(out=xc_sb[0][:], in_=xcT[0:128, :])
        nc.sync.dma_start(out=xc_sb[1][:], in_=xcT[128:256, :])
        nc.sync.dma_start(out=wq_sb[:],
                          in_=wq.rearrange("(ct p) d -> p ct d", p=128))
        nc.sync.dma_start(out=xc_sb[2][:], in_=xcT[256:384, :])
        nc.sync.dma_start(out=xc_sb[3][:], in_=xcT[384:512, :])
        nc.sync.dma_start(out=xp_sb[0][:], in_=xpT[0:128, :])
        nc.sync.dma_start(out=xp_sb[1][:], in_=xpT[128:256, :])
        nc.sync.dma_start(out=xp_sb[2][:], in_=xpT[256:384, :])

        # ---- constant / zero-pad memsets on idle engines (Pool + DVE),
        # most-urgent first (kt/qt pads gate the first S^T) ----
        nc.vector.memset(kt_p[0][64:128, :], 0.0)
        nc.vector.memset(qt_p[0][64:128, :], 0.0)
        nc.gpsimd.memset(kt_p[1][0:64, :], 0.0)
        nc.gpsimd.memset(qt_p[1][0:64, :], 0.0)
        nc.gpsimd.memset(z_sb[0][:], 0.0)
        nc.gpsimd.memset(z_sb[1][:], 0.0)
        nc.gpsimd.memset(otn[0][64:128, :], 0.0)
        nc.gpsimd.memset(otn[1][64:128, :], 0.0)
        nc.gpsimd.memset(wp0_sb[64:128, :], 0.0)
        nc.gpsimd.memset(wp1_sb[64:128, :], 0.0)
        nc.vector.memset(ones_pad[:], 0.0)
        nc.vector.memset(ones_pad[0:1, :], 1.0)
        for g in range(2):
            nc.vector.memset(vones[g][:, :, 64:65], 1.0)
            nc.vector.memset(vones[g][:, :, 129:130], 1.0)

        # ---- KT, V, QT on the PE (matches data-arrival order).  PSUM
        # bank choreography: kt_ps's pool closes before qt_ps opens so QT
        # reuses KT's banks; v_ps holds its own; s_t/pv later grab the
        # freed kt/qt banks + fresh ones and do NOT alias v_ps, so the
        # first S^T needs only the kt/qt evacuations, not V's. ----
        # right-side pool packing rbc (bank-aligned, first) with the two
        # V ping-pong tiles: 2048+512+512B -> 2 banks
        v_stack = ExitStack()
        v_pool = v_stack.enter_context(
            tc.tile_pool(name="vr_ps", bufs=1, space="PSUM", side="right"))
        rbc_ps = v_pool.tile([64, 512], F32, tag="rbcps", name="rbcps")
        v_ps = [v_pool.tile([128, 128], F32, tag="vps", name="vps")]
        with ExitStack() as kctx:
            kt_pool = kctx.enter_context(
                tc.tile_pool(name="kt_ps", bufs=1, space="PSUM"))
            kt_ps = [kt_pool.tile([128, 512], F32, tag=f"ktps{nh}",
                                  name=f"ktps{nh}") for nh in range(2)]
            for nh in range(2):
                for ct in range(CT_N):
                    nc.tensor.matmul(
                        out=kt_ps[nh][:],
                        lhsT=wk_sb[:, ct, :],
                        rhs=xc_sb[ct][:, nh * 512:(nh + 1) * 512],
                        start=(ct == 0), stop=(ct == CT_N - 1))
            for nh in range(2):
                o = nh * 512
                if nh == 0:
                    nc.vector.tensor_copy(out=kt_p[0][0:64, o:o + 512],
                                          in_=kt_ps[nh][0:64, :])
                    nc.scalar.copy(out=kt_p[1][64:128, o:o + 512],
                                   in_=kt_ps[nh][64:128, :])
                else:
                    nc.scalar.copy(out=kt_p[0][0:64, o:o + 512],
                                   in_=kt_ps[nh][0:64, :])
                    nc.vector.tensor_copy(out=kt_p[1][64:128, o:o + 512],
                                          in_=kt_ps[nh][64:128, :])

        for kt in range(KT_N):
            vt = v_ps[0]
            for ct in range(CT_N):
                nc.tensor.matmul(
                    out=vt[:],
                    lhsT=xc_sb[ct][:, kt * 128:(kt + 1) * 128],
                    rhs=wv_sb[:, ct, :],
                    start=(ct == 0), stop=(ct == CT_N - 1))
            dst = vones[kt // 4][:, kt % 4, :].rearrange(
                "p (g s) -> p g s", g=2)[:, :, 0:64]
            vsrc = vt[:].rearrange("p (g s) -> p g s", g=2)
            # alternate engines so the qt evacuations aren't queued behind
            # all eight vones copies on the DVE
            if kt % 2 == 0:
                nc.vector.tensor_copy(out=dst, in_=vsrc)
            else:
                nc.scalar.copy(out=dst, in_=vsrc)

        with ExitStack() as qctx:
            qt_pool = qctx.enter_context(
                tc.tile_pool(name="qt_ps", bufs=1, space="PSUM"))
            qt_ps = [qt_pool.tile([128, 512], F32, tag=f"qtps{nh}",
                                  name=f"qtps{nh}") for nh in range(2)]
            # ct order matches xp chunk arrival (xp3 rides the early
            # scalar queue, xp0-2 close the sync stream)
            qt_order = [3, 0, 1, 2]
            for nh in range(2):
                for j, ct in enumerate(qt_order):
                    nc.tensor.matmul(
                        out=qt_ps[nh][:],
                        lhsT=wq_sb[:, ct, :],
                        rhs=xp_sb[ct][:, nh * 512:(nh + 1) * 512],
                        start=(j == 0), stop=(j == CT_N - 1))
            for nh in range(2):
                o = nh * 512
                if nh == 0:
                    nc.vector.tensor_copy(out=qt_p[0][0:64, o:o + 512],
                                          in_=qt_ps[nh][0:64, :])
                    nc.scalar.copy(out=qt_p[1][64:128, o:o + 512],
                                   in_=qt_ps[nh][64:128, :])
                else:
                    nc.scalar.copy(out=qt_p[0][0:64, o:o + 512],
                                   in_=qt_ps[nh][0:64, :])
                    nc.vector.tensor_copy(out=qt_p[1][64:128, o:o + 512],
                                          in_=qt_ps[nh][64:128, :])

        # ---- attention, head-sequential: S^T -> exp -> PV (+Z via ones
        # column).  h0's PV result is staged out of PSUM immediately so
        # h1's PV (in-order PE queue!) only waits ~1 iteration; h0's
        # normalizer then runs from SBUF on the idle DVE under h1's exp
        # stream.  h1's tail reads pv directly (nothing waits on it). ----
        with ExitStack() as actx:
            # right stack: vr(2 banks, stays open: rbc + v ping-pong) +
            # pv(2); left stack: s_t(4) reusing kt/qt's banks + 2 fresh.
            pv_pool = actx.enter_context(
                tc.tile_pool(name="pv_ps", bufs=1, space="PSUM",
                             side="right"))
            pv = pv_pool.tile([65, SEQ], F32, tag="pv", name="pv")
            s_stack = ExitStack()
            s_pool = s_stack.enter_context(
                tc.tile_pool(name="s_ps", bufs=1, space="PSUM"))
            s_t = [s_pool.tile([128, SEQ], F32, tag=f"st{i}", name=f"st{i}")
                   for i in range(2)]
            items = [(h, kt) for h in range(2) for kt in range(KT_N)]

            def emit_st(i):
                h, kt = items[i]
                s = s_t[i % 2]
                for nh in range(2):
                    nc.tensor.matmul(
                        out=s[:, nh * 512:(nh + 1) * 512],
                        lhsT=kt_p[h][:, kt * 128:(kt + 1) * 128],
                        rhs=qt_p[h][:, nh * 512:(nh + 1) * 512],
                        start=True, stop=True)

            # Normalizer tails.  Both heads use the ACT Ln -> broadcast ->
            # Exp(-x) chain (the DVE reciprocal is ~6.4 cyc/elem AND the
            # tile scheduler mis-models it as ~1us, wrecking the PE
            # order).  h0's chain is dribbled into the exp stream one op
            # per iteration (costs ~2.4us of ACT there, but no stalls);
            # h1's runs at stream end when ACT goes idle.  h0's pv is
            # staged to SBUF first so h1's PV accumulation (in-order PE
            # queue) isn't blocked behind h0's consumers.
            def tail0_step(step):
                if step == 0:
                    nc.vector.tensor_copy(out=pv_stage[:], in_=pv[:])
                elif step == 1:
                    nc.scalar.activation(out=z_sb[0][0:1, :],
                                         in_=pv_stage[64:65, :], func=Ln)
                elif step in (2, 3):
                    o = (step - 2) * 512
                    nc.tensor.matmul(out=rbc_ps[:], lhsT=ones_pad[:],
                                     rhs=z_sb[0][:, o:o + 512],
                                     start=True, stop=True)
                    nc.scalar.activation(out=rbc_sb[0][:, o:o + 512],
                                         in_=rbc_ps[:], func=Exp,
                                         scale=-1.0)
                elif step == 4:
                    for nh in range(2):
                        o = nh * 512
                        nc.vector.tensor_mul(out=otn[0][0:64, o:o + 512],
                                             in0=pv_stage[0:64, o:o + 512],
                                             in1=rbc_sb[0][:, o:o + 512])

            emit_st(0)
            for i, (h, kt) in enumerate(items):
                if i + 1 < len(items):
                    emit_st(i + 1)
                p = p_t[i % P_DEPTH]
                nc.scalar.activation(out=p[:], in_=s_t[i % 2][:], func=Exp,
                                     scale=float(SCALE))
                for nh in range(2):
                    nc.tensor.matmul(
                        out=pv[:, nh * 512:(nh + 1) * 512],
                        lhsT=vones[kt // 4][:, kt % 4, h * 65:h * 65 + 65],
                        rhs=p[:, nh * 512:(nh + 1) * 512],
                        start=(kt == 0), stop=(kt == KT_N - 1))
                if 7 <= i <= 11:
                    tail0_step(i - 7)
            s_stack.close()

            # ---- h1 tail + projection; out_ps (3-deep) and a second rbc
            # tile reuse the s_t banks (s_pool closed after the last exp)
            with ExitStack() as tctx:
                out_pool = tctx.enter_context(
                    tc.tile_pool(name="out_ps", bufs=1, space="PSUM"))
                out_ps = [out_pool.tile([128, C], F32, tag=f"ops{i}",
                                        name=f"ops{i}") for i in range(3)]
                rbc2_pool = tctx.enter_context(
                    tc.tile_pool(name="rbc2_ps", bufs=1, space="PSUM"))
                rbc2_ps = rbc2_pool.tile([64, 512], F32, tag="rbc2ps",
                                         name="rbc2ps")

                # h1's normalizer at stream end on the then-idle ACT; the
                # two halves use separate rbc tiles so the second bcast
                # doesn't wait for the first Exp read
                for nh in range(2):
                    o = nh * 512
                    nc.scalar.activation(out=z_sb[1][0:1, o:o + 512],
                                         in_=pv[64:65, o:o + 512],
                                         func=Ln)
                for nh in range(2):
                    o = nh * 512
                    rb = rbc_ps if nh == 0 else rbc2_ps
                    nc.tensor.matmul(out=rb[:], lhsT=ones_pad[:],
                                     rhs=z_sb[1][:, o:o + 512],
                                     start=True, stop=True)
                    nc.scalar.activation(out=rbc_sb[1][:, o:o + 512],
                                         in_=rb[:], func=Exp,
                                         scale=-1.0)
                for nh in range(2):
                    o = nh * 512
                    nc.vector.tensor_mul(out=otn[1][0:64, o:o + 512],
                                         in0=pv[0:64, o:o + 512],
                                         in1=rbc_sb[1][:, o:o + 512])

                for qt in range(KT_N):
                    q = qt * 128
                    ot = out_ps[qt % 3]
                    nc.tensor.matmul(out=ot[:],
                                     lhsT=otn[0][:, q:q + 128],
                                     rhs=wp0_sb[:], start=True,
                                     stop=False)
                    nc.tensor.matmul(out=ot[:],
                                     lhsT=otn[1][:, q:q + 128],
                                     rhs=wp1_sb[:], start=False,
                                     stop=True)
                    # alternate evac engines so the proj pipeline is
                    # PE-paced, not evacuation-paced (Pool can't read PSUM)
                    o16 = o16_t[qt % 4]
                    if qt % 2 == 0:
                        nc.vector.tensor_copy(out=o16[:], in_=ot[:])
                    else:
                        nc.scalar.copy(out=o16[:], in_=ot[:])
                    eng = nc.sync if qt % 2 == 0 else nc.scalar
                    eng.dma_start(
                        out=out[qt * 128:(qt + 1) * 128, :], in_=o16[:])
        v_stack.close()


def _get_program():
    global _PROG
    if _PROG is None:
        _PROG = _build_program()
    return _PROG


def _shard_inputs(x_pred, x_ctx, ctx_mask, Wq, Wkv, Wproj):
    """Build the 8 per-core input maps (host-side sharding)."""
    ctx_mask = np.asarray(ctx_mask).astype(bool)
    pidx = np.nonzero(~ctx_mask.reshape(-1))[0]
    cidx = np.nonzero(ctx_mask.reshape(-1))[0]
    pm = [np.where(pidx // T == b)[0] for b in range(B)]
    cm = [np.where(cidx // T == b)[0] for b in range(B)]
    for b in range(B):
        assert len(pm[b]) == T_CTX and len(cm[b]) == T_CTX, (
            "kernel compiled for T_CTX ctx/pred slots per batch row")

    xpT_b, xcT_b = [], []
    for b in range(B):
        Xp = x_pred[pm[b]].reshape(SEQ, C)
        Xc = x_ctx[cm[b]].reshape(SEQ, C)
        xpT_b.append(np.ascontiguousarray(Xp.T).astype(np.float16))
        xcT_b.append(np.ascontiguousarray(Xc.T).astype(np.float16))

    wq16 = Wq.astype(np.float16)
    wk16 = Wkv[:, :C].astype(np.float16)
    wv16 = Wkv[:, C:].astype(np.float16)
    wp16 = Wproj.astype(np.float16)

    in_maps = []
    for c in range(NCORE):
        b, hp = divmod(c, 4)
        hc = hp * 128
        in_maps.append({
            "xpT": xpT_b[b],
            "xcT": xcT_b[b],
            "wq": np.ascontiguousarray(wq16[:, hc:hc + 128]),
            "wk": np.ascontiguousarray(wk16[:, hc:hc + 128]),
            "wv": np.ascontiguousarray(wv16[:, hc:hc + 128]),
            "wp": np.ascontiguousarray(wp16[hc:hc + 128, :]),
        })
    return in_maps, pm


def _unshard_output(results, pm, bproj, dtype):
    full = np.zeros((B * T_CTX, N, C), dtype)
    for b in range(B):
        acc = results[4 * b]["out"].astype(np.float64)
        for j in range(1, 4):
            acc = acc + results[4 * b + j]["out"]
        acc = (acc + bproj).astype(dtype)
        full[pm[b]] = acc.reshape(T_CTX, N, C)
    return full


def run(inputs, trace=False, **kwargs):
    """Run the SPMD kernel; returns (full_output, BassKernelResults)."""
    from concourse.bass_utils import run_bass_kernel_spmd

    nc = _get_program()
    in_maps, pm = _shard_inputs(inputs["x_pred"], inputs["x_ctx"],
                                inputs["ctx_mask"], inputs["Wq"],
                                inputs["Wkv"], inputs["Wproj"])
    res = run_bass_kernel_spmd(nc, in_maps, list(range(NCORE)), trace=trace,
                               **kwargs)
    out = _unshard_output(res.results, pm, np.asarray(inputs["bproj"]),
                          np.asarray(inputs["x_pred"]).dtype)
    return out, res


def kernel(x_pred, x_ctx, ctx_mask, Wq, Wkv, Wproj, bproj):
    out, _ = run(dict(x_pred=np.asarray(x_pred), x_ctx=np.asarray(x_ctx),
                      ctx_mask=np.asarray(ctx_mask), Wq=np.asarray(Wq),
                      Wkv=np.asarray(Wkv), Wproj=np.asarray(Wproj),
                      bproj=np.asarray(bproj)))
    return out
